# revision 1
# baseline (speedup 1.0000x reference)
"""AdjStackAttentionWeights kernel for 8 Trainium2 NeuronCores.

Computation: masked BatchNorm (training-mode stats over masked rows of the
whole tensor), normalize, 2-layer MLP (32 -> 64 relu -> 16), mask the output.

Strategy:
  - Shard batch dim b across the 8 cores (data parallel).
  - Host premultiplies x by the mask and lays the result out in the exact
    [128, 2048] SBUF tile layout the device consumes (partition p = q*32+s
    holds feature s of row-quarter q), so every device DMA is a fully
    contiguous 1 MiB read / 512 KiB write.
  - pass 1: bn_stats over [128,512] tile slices -> masked sum / sumsq per
            feature -> AllReduce of [32,2] across cores. The first KEEP
            megatiles stay resident in SBUF and are reused by pass 2.
  - fold:   BN scale folded into W1 (W1' = diag(s)@W1); shift becomes a
            per-partition bias b1' = (beta - mean*s)@W1 + b1 that rides the
            PSUM->SBUF relu copy.
  - pass 2: hT = relu(W1'.T @ xmT + b1'); outT = W2.T @ hT + b2, streamed
            through the PE with 4-way tile_position packing.
  - Matmul operands use float32r (fp32 rounded to 12-bit-less mantissa),
    which streams 4x faster through the PE than plain fp32; the host
    pre-rounds the uploads.  Set USE_F32R = False to fall back to full fp32.
  - Rows with m=0 produce garbage (+b2); the host zeroes them (the reference
    multiplies by the mask anyway).
"""

import numpy as np

B, NN, S, H, HEADS = 8, 512, 32, 64, 16
R_FULL = NN * NN  # 262144 rows per core
FD = 512          # free-dim elements per quarter tile
QS = 4            # quarters stacked on the partition axis
ST = QS * FD      # 2048 rows per supertile
MG = 4            # supertiles per megatile (1 MiB DMA granularity)
NCORES = 8
BN_EPS = 1e-5
USE_F32R = True
USE_BF16_STATS = True   # read the non-cached pass-1 stream in bf16
KEEP = 14         # megatiles kept resident in SBUF between the passes

_NC_CACHE = {}


def build_nc(ncores=NCORES, rows=R_FULL, keep=KEEP):
    """Build (and bacc-compile) the SPMD bass program for one core."""
    import concourse.bass as bass
    import concourse.tile as tile
    from concourse import bacc, mybir

    f32 = mybir.dt.float32
    fmm = mybir.dt.float32r if USE_F32R else f32
    T = rows // ST          # supertiles
    TG = T // MG            # megatiles
    keep = min(keep, TG)
    assert TG * MG == T and T * ST == rows

    nc = bacc.Bacc("TRN2", target_bir_lowering=False, debug=False,
                   num_devices=ncores)

    bf16 = mybir.dt.bfloat16
    f16 = mybir.dt.float16
    xmt = nc.dram_tensor("xmt", [TG, 128, MG * FD], fmm, kind="ExternalInput")
    nbf = max(TG - keep, 1)
    xbt = nc.dram_tensor("xbt", [nbf, 128, MG * FD], bf16, kind="ExternalInput")
    # w1f: two stacked copies of blockdiag(W1, W1) [64, 128]
    w1f = nc.dram_tensor("w1f", [128, 2 * H], fmm, kind="ExternalInput")
    w2t = nc.dram_tensor("w2t", [128, 2 * HEADS], f16, kind="ExternalInput")
    w1r = nc.dram_tensor("w1r", [S, H], f32, kind="ExternalInput")  # raw W1
    # constant selector matrices for PE-side partition reshuffles
    qmat = nc.dram_tensor("qmat", [128, S], f32, kind="ExternalInput")
    bm32 = nc.dram_tensor("bm32", [S, 128], f32, kind="ExternalInput")
    bm64 = nc.dram_tensor("bm64", [H, 128], f32, kind="ExternalInput")
    svec = nc.dram_tensor("svec", [S, 4], f32, kind="ExternalInput")
    b1c = nc.dram_tensor("b1c", [H, 1], f32, kind="ExternalInput")
    b2t = nc.dram_tensor("b2t", [128, 1], f32, kind="ExternalInput")
    # padded fp16 store: full 128 partitions (pads included) so the store DMA
    # engages all ports with one contiguous 512 KiB write per megatile
    out = nc.dram_tensor("out", [TG, 128, MG * FD], f16,
                         kind="ExternalOutput")

    xview = xmt.ap()
    oview = out.ap()

    with tile.TileContext(nc) as tc:
        with (
            tc.tile_pool(name="wpool", bufs=1) as wpool,
            tc.tile_pool(name="glue", bufs=1) as glue,
            tc.tile_pool(name="bn", bufs=1) as bnpool,
            tc.tile_pool(name="res", bufs=1) as respool,
            tc.tile_pool(name="stream", bufs=6) as stpool,
            tc.tile_pool(name="stream2", bufs=5) as stpool2,
            tc.tile_pool(name="h", bufs=4) as hpool,
            tc.tile_pool(name="o", bufs=2) as opool,
            tc.tile_pool(name="psum", bufs=2, space="PSUM") as pspool,
            tc.tile_pool(name="psumc", bufs=2, space="PSUM") as pspoolc,
            tc.tile_pool(name="psum1", bufs=1, space="PSUM") as pspool1,
            tc.tile_pool(name="dram", bufs=1, space="DRAM") as dpool,
        ):
            # ---- resident weights/constants -------------------------------
            w1sb = wpool.tile([128, 2 * H], fmm)      # 2x blockdiag(W1, W1)
            nc.sync.dma_start(w1sb[:], w1f[:])
            w2sb = wpool.tile([128, 2 * HEADS], f16)  # 2 stacked [64,32] pads
            nc.sync.dma_start(w2sb[:], w2t[:])
            b2sb = wpool.tile([128, 1], f32)
            nc.sync.dma_start(b2sb[:], b2t[:])
            w1rsb = glue.tile([S, H], f32)
            nc.sync.dma_start(w1rsb[:], w1r[:])
            qmsb = glue.tile([128, S], f32)
            nc.sync.dma_start(qmsb[:], qmat[:])
            b32sb = glue.tile([S, 128], f32)
            nc.sync.dma_start(b32sb[:], bm32[:])
            b64sb = glue.tile([H, 128], f32)
            nc.sync.dma_start(b64sb[:], bm64[:])
            svsb = glue.tile([S, 4], f32)
            nc.sync.dma_start(svsb[:], svec[:])
            b1sb = glue.tile([H, 1], f32)
            nc.sync.dma_start(b1sb[:], b1c[:])

            # ---- pass 1: bn_stats over all tiles --------------------------
            bnbuf = bnpool.tile([128, 6 * T], f32)
            xtiles = {}
            for g in range(TG):
                if g < keep:
                    st_tile = respool.tile([128, MG * FD], fmm, tag=f"res{g}")
                    xtiles[g] = st_tile
                    nc.sync.dma_start(st_tile[:], xview[g])
                    src = st_tile[:].bitcast(f32)
                elif USE_BF16_STATS:
                    st_tile = stpool.tile([128, MG * FD], bf16, tag="stream")
                    nc.sync.dma_start(st_tile[:], xbt.ap()[g - keep])
                    src = st_tile[:]
                else:
                    st_tile = stpool.tile([128, MG * FD], fmm, tag="stream")
                    nc.sync.dma_start(st_tile[:], xview[g])
                    src = st_tile[:].bitcast(f32)
                for u in range(MG):
                    t = g * MG + u
                    nc.vector.bn_stats(bnbuf[:, 6 * t:6 * t + 6],
                                       src[:, FD * u:FD * u + FD])

            cached = list(range(keep))
            streamed = list(range(keep, TG))

            # convert (count, mean, count*var) x {even, odd} into sums
            # bnbuf view [128, T, 6]; means at cols 1,4; cvars at cols 2,5
            bnv = bnbuf[:].rearrange("p (t k) -> p t k", k=6)
            means = bnv[:, :, 1:5:3]   # [128, T, 2] (cols 1 and 4)
            cvars = bnv[:, :, 2:6:3]   # [128, T, 2] (cols 2 and 5)
            half = float(FD // 2)

            msq = glue.tile([128, 2 * T], f32)
            nc.vector.tensor_mul(msq[:], means, means)
            sum_means = glue.tile([128, 1], f32)
            nc.vector.tensor_reduce(sum_means[:], means,
                                    axis=mybir.AxisListType.XY,
                                    op=mybir.AluOpType.add)
            sum_msq = glue.tile([128, 1], f32)
            nc.vector.tensor_reduce(sum_msq[:], msq[:],
                                    axis=mybir.AxisListType.X,
                                    op=mybir.AluOpType.add)
            sum_cv = glue.tile([128, 1], f32)
            nc.vector.tensor_reduce(sum_cv[:], cvars,
                                    axis=mybir.AxisListType.XY,
                                    op=mybir.AluOpType.add)
            partials = glue.tile([128, 2], f32)
            # sum(x) = half * sum(means)
            nc.vector.tensor_scalar_mul(partials[:, 0:1], sum_means[:], half)
            # sum(x^2) = half * sum(means^2) + sum(count*var)
            nc.vector.tensor_scalar(partials[:, 1:2], sum_msq[:], half,
                                    sum_cv[:], op0=mybir.AluOpType.mult,
                                    op1=mybir.AluOpType.add)

            # fold the 4 partition quarters on the PE: local = Q.T @ partials
            ps_st = pspool1.tile([S, 2], f32, tag="psg")
            nc.tensor.matmul(ps_st[:], qmsb[:], partials[:], start=True,
                             stop=True, tile_position=(0, 0))
            local = glue.tile([S, 2], f32)
            nc.vector.tensor_copy(local[:], ps_st[:])

            # ---- AllReduce of [32,2] masked sums across cores -------------
            ar_in = dpool.tile([S, 2], f32)
            ar_out = dpool.tile([S, 2], f32)
            nc.gpsimd.dma_start(ar_in[:], local[:])
            nc.gpsimd.collective_compute(
                "AllReduce",
                mybir.AluOpType.add,
                replica_groups=[list(range(ncores))],
                ins=[ar_in.opt()],
                outs=[ar_out.opt()],
            )
            gst = glue.tile([S, 2], f32)
            nc.gpsimd.dma_start(gst[:], ar_out[:])

            # ---- fold stats into weights ----------------------------------
            mean = glue.tile([S, 1], f32)
            nc.vector.tensor_mul(mean[:], gst[:, 0:1], svsb[:, 2:3])
            ex2 = glue.tile([S, 1], f32)
            nc.vector.tensor_mul(ex2[:], gst[:, 1:2], svsb[:, 2:3])
            var = glue.tile([S, 1], f32)
            nc.vector.tensor_mul(var[:], mean[:], mean[:])
            nc.vector.tensor_sub(var[:], ex2[:], var[:])
            nc.vector.tensor_scalar_add(var[:], var[:], BN_EPS)
            recip = glue.tile([S, 1], f32)
            nc.vector.reciprocal(recip[:], var[:])
            rstd = glue.tile([S, 1], f32)
            nc.scalar.activation(rstd[:], recip[:],
                                 mybir.ActivationFunctionType.Sqrt)
            sg = glue.tile([S, 1], f32)
            nc.vector.tensor_mul(sg[:], rstd[:], svsb[:, 0:1])      # s = gamma*rstd
            tv = glue.tile([S, 1], f32)
            nc.vector.tensor_mul(tv[:], mean[:], sg[:])
            nc.vector.tensor_sub(tv[:], svsb[:, 1:2], tv[:])        # t = beta-mean*s
            # b1' = W1.T @ t + b1  (plain-fp32 matmul on the raw W1 copy)
            b1p = pspool1.tile([H, 1], f32, tag="psg")
            nc.tensor.matmul(b1p[:], w1rsb[:], tv[:], start=True,
                             stop=True, tile_position=(0, 0))
            b1f = glue.tile([H, 1], f32)
            nc.vector.tensor_add(b1f[:], b1p[:], b1sb[:])

            # broadcast b1' and s to [128,1] via PE selector matmuls
            ps_b = pspool1.tile([128, 1], f32, tag="psg")
            nc.tensor.matmul(ps_b[:], b64sb[:], b1f[:], start=True,
                             stop=True, tile_position=(0, 0))
            bias128 = wpool.tile([128, 1], f32)
            nc.vector.tensor_copy(bias128[:], ps_b[:])
            ps_s = pspool1.tile([128, 1], f32, tag="psg")
            nc.tensor.matmul(ps_s[:], b32sb[:], sg[:], start=True,
                             stop=True, tile_position=(0, 0))
            s4 = wpool.tile([128, 1], f32)
            nc.vector.tensor_copy(s4[:], ps_s[:])
            # scale all four W1 copies in place: W1' = diag(s) @ W1
            nc.vector.tensor_scalar(w1sb[:], w1sb[:], s4[:], None,
                                    op0=mybir.AluOpType.mult)

            # ---- pass 2: the MLP ------------------------------------------
            relu = mybir.ActivationFunctionType.Relu
            ident = mybir.ActivationFunctionType.Identity
            # interleave cached and streamed megatiles so the DMA stream is
            # busy from the moment the collective completes
            order = []
            a = b = 0
            while a < len(cached) or b < len(streamed):
                if b < len(streamed) and (a >= len(cached)
                                          or b * len(cached) < a * len(streamed)):
                    order.append(streamed[b]); b += 1
                else:
                    order.append(cached[a]); a += 1
            for g in order:
                if g in xtiles:
                    xt = xtiles[g]
                else:
                    xt = stpool2.tile([128, MG * FD], fmm, tag="stream2")
                    nc.sync.dma_start(xt[:], xview[g])
                omega = opool.tile([128, MG * FD], f16)
                for u in range(MG):
                    t = g * MG + u
                    xs = xt[:, FD * u:FD * u + FD]
                    psA = pspool.tile([128, FD], f32, tag="psA")
                    psB = pspool.tile([128, FD], f32, tag="psB")
                    # paired mm1: blockdiag(W1',W1') handles two quarters per
                    # column; outputs land exactly like the 4-way version
                    nc.tensor.matmul(psA[:], w1sb[0:2 * S, :], xs[0:2 * S, :],
                                     start=True, stop=True,
                                     tile_position=(0, 0))
                    nc.tensor.matmul(psB[:], w1sb[2 * S:128, :],
                                     xs[2 * S:128, :],
                                     start=True, stop=True,
                                     tile_position=(64, 0))
                    hA = hpool.tile([128, FD], f16, tag="hA")
                    hB = hpool.tile([128, FD], f16, tag="hB")
                    # relu(z + b1'): alternate engines to balance ACT/DVE
                    if t % 2 == 0:
                        nc.scalar.activation(hA[:], psA[:], relu,
                                             bias=bias128[:])
                        nc.vector.tensor_scalar(hB[:], psB[:], bias128[:], 0.0,
                                                op0=mybir.AluOpType.add,
                                                op1=mybir.AluOpType.max)
                    else:
                        nc.vector.tensor_scalar(hA[:], psA[:], bias128[:], 0.0,
                                                op0=mybir.AluOpType.add,
                                                op1=mybir.AluOpType.max)
                        nc.scalar.activation(hB[:], psB[:], relu,
                                             bias=bias128[:])
                    psC = pspoolc.tile([128, FD], f32, tag="psC")
                    for q in range(QS):
                        hs = (hA, hB)[q // 2]
                        o = q % 2
                        nc.tensor.matmul(psC[32 * q:32 * q + 32, :],
                                         w2sb[64 * o:64 * o + 64, :],
                                         hs[64 * o:64 * o + 64, :],
                                         start=True, stop=True,
                                         tile_position=(64 * o, 32 * q))
                    od = omega[:, FD * u:FD * u + FD]
                    if t % 3 == 2:
                        nc.vector.tensor_scalar(od, psC[:], b2sb[:], None,
                                                op0=mybir.AluOpType.add)
                    else:
                        nc.scalar.activation(od, psC[:], ident, bias=b2sb[:])
                nc.sync.dma_start(oview[g], omega[:])

    nc.compile()
    return nc


def _get_nc(ncores, rows):
    key = (ncores, rows)
    if key not in _NC_CACHE:
        _NC_CACHE[key] = build_nc(ncores, rows)
    return _NC_CACHE[key]


def _round_f32r(a):
    """Round fp32 array to float32r (low 12 mantissa bits dropped, RTN)."""
    if not USE_F32R:
        return a
    u = a.view(np.uint32)
    r = (u + np.uint32(0x800)) & np.uint32(0xFFFFF000)
    return r.view(np.float32)


def _tile_layout(xm, rows):
    """[rows, S] masked input -> [TG, 128, MG*FD] device tile layout."""
    T = rows // ST
    TG = T // MG
    # row r = ((g*MG + u)*QS + q)*FD + j ; partition p = q*32 + s
    v = xm.reshape(TG, MG, QS, FD, S)          # [g, u, q, j, s]
    v = v.transpose(0, 2, 4, 1, 3)             # [g, q, s, u, j]
    return np.ascontiguousarray(v).reshape(TG, 128, MG * FD)


def make_in_maps(stacks, mask, gamma, beta, W1, b1, W2, b2, ncores=NCORES,
                 rows=R_FULL):
    """Host-side prep: per-core input dicts (layout transforms only)."""
    stacks = np.asarray(stacks)
    mask = np.asarray(mask)
    cnt = max(float(np.asarray(mask, np.float64).sum()), 1.0)
    inv_cnt = np.float32(1.0 / np.float32(cnt))

    svec = np.zeros((S, 4), np.float32)
    svec[:, 0] = np.asarray(gamma, np.float32)
    svec[:, 1] = np.asarray(beta, np.float32)
    svec[:, 2] = inv_cnt

    qm = np.zeros((128, S), np.float32)
    qm[np.arange(128), np.arange(128) % S] = 1.0
    b32 = np.ascontiguousarray(qm.T)              # [32, 128]
    b64 = np.zeros((H, 128), np.float32)
    b64[np.arange(128) % H, np.arange(128)] = 1.0

    w1np = np.asarray(W1, np.float32)
    bd = np.zeros((2 * S, 2 * H), np.float32)     # blockdiag(W1, W1)
    bd[:S, :H] = w1np
    bd[S:, H:] = w1np
    w1f = _round_f32r(np.tile(bd, (2, 1)))        # [128, 128]
    w2pad = np.zeros((64, 2 * HEADS), np.float16)
    w2pad[:, :HEADS] = np.asarray(W2, np.float32).astype(np.float16)
    w2t = np.tile(w2pad, (2, 1))                  # [128, 32] fp16
    b1c = np.asarray(b1, np.float32).reshape(H, 1)
    b2t = np.zeros((128, 1), np.float32)
    for q in range(QS):
        b2t[32 * q:32 * q + HEADS, 0] = np.asarray(b2, np.float32)

    import ml_dtypes
    T = rows // ST
    TG = T // MG
    keep = min(KEEP, TG)
    in_maps = []
    for c in range(ncores):
        x = np.asarray(stacks[c], np.float32).reshape(-1, S)[:rows]
        m = np.asarray(mask[c]).reshape(-1)[:rows]
        xm = _round_f32r(x * m[:, None].astype(np.float32))
        xt = _tile_layout(xm, rows)
        xb = xt[keep:] if TG > keep else xt[:1]
        in_maps.append({
            "xmt": xt, "xbt": np.ascontiguousarray(xb.astype(ml_dtypes.bfloat16)),
            "w1f": w1f, "w2t": w2t,
            "w1r": w1np, "svec": svec, "b1c": b1c, "b2t": b2t,
            "qmat": qm, "bm32": b32, "bm64": b64,
        })
    return in_maps


def assemble_output(results, mask, ncores=NCORES, rows=R_FULL):
    T = rows // ST
    TG = T // MG
    outs = []
    for c in range(ncores):
        o = results[c]["out"].astype(np.float32)    # [TG, 128, MG*FD] fp16
        o = o.reshape(TG, QS, 32, MG, FD)[:, :, :HEADS]   # [g, q, h, u, j]
        o = o.transpose(0, 3, 1, 4, 2)              # [g, u, q, j, h]
        o = np.ascontiguousarray(o).reshape(rows, HEADS)
        m = np.asarray(mask[c]).reshape(-1)[:rows]
        outs.append(o * m[:, None].astype(np.float32))
    return np.stack(outs)                           # [ncores, rows, 16]


def kernel(stacks, mask, gamma, beta, W1, b1, W2, b2):
    from concourse.bass_utils import run_bass_kernel_spmd

    nc = _get_nc(NCORES, R_FULL)
    in_maps = make_in_maps(stacks, mask, gamma, beta, W1, b1, W2, b2)
    res = run_bass_kernel_spmd(nc, in_maps, list(range(NCORES)))
    out = assemble_output(res.results, mask)
    return out.reshape(B, NN, NN, HEADS)



# revision 8
# speedup vs baseline: 1.5632x; 1.5632x over previous
"""AdjStackAttentionWeights kernel for 8 Trainium2 NeuronCores.

Computation: masked BatchNorm (training-mode stats over masked rows of the
whole tensor), normalize, 2-layer MLP (32 -> 64 relu -> 16), mask the output.

Strategy (v2 — mask compaction + fully SBUF-resident single pass):
  - Shard batch dim b across the 8 cores (data parallel).
  - ~50% of rows are masked out and produce zero output. The host gathers
    only the masked-in rows per core, pads to a pair-of-supertiles multiple
    (4096 rows), and uploads fp16 in the exact [128, 1024] SBUF tile layout
    (partition p = q*32 + s holds feature s of row-quarter q; free dim is
    two 512-row supertiles). Roughly halves both DMA and PE work.
  - The whole compacted input (~8.5 MiB/core) stays RESIDENT in SBUF: one
    HBM read total.
  - pass 1 (overlapped with the input DMA): masked sum / sum-of-squares per
    feature, split across three engines: even pairs DVE bn_stats, odd pairs
    ACT Square+accum (sumsq) + GpSimd tensor_reduce (sum).
    -> AllReduce of [32,2] partial sums across the 8 cores.
  - fold: BN scale folded into W1 (W1' = diag(s)@W1); shift becomes a
    per-partition bias b1' = (beta - mean*s)@W1 + b1 applied during relu.
  - pass 2 (from SBUF): per supertile u, mm1 computes all 4 quarters' h into
    one [128, 1024] PSUM tile (two 64-contraction blockdiag(W1',W1')
    matmuls); one relu+bias copy -> fp16 h (ACT for even u, DVE for odd);
    mm2 uses a [128, 32] hidden-blockdiag W2 so ONE matmul yields two
    quarters' 16 heads -> compact psC [64, 1024] per pair; one DVE copy ->
    fp16 omega; GpSimd-issued DMA out. PE-bound at ~1 cycle/row fp16.
  - b2 and the output mask/scatter are applied on the host (b2 is a
    constant [16] broadcast, same class of host work as the mask multiply).
"""

import numpy as np

B, NN, S, H, HEADS = 8, 512, 32, 64, 16
R_FULL = NN * NN   # 262144 rows per core before compaction
FD = 512           # free-dim elements per supertile quarter
QS = 4             # quarters stacked on the partition axis
ST = QS * FD       # 2048 rows per supertile
PAIR = 2 * ST      # 4096 rows per resident [128, 1024] tile
NCORES = 8
BN_EPS = 1e-5

_NC_CACHE = {}


def build_nc(ncores=NCORES, npairs=33):
    """Build (and bacc-compile) the SPMD bass program for one core."""
    import concourse.bass as bass
    import concourse.tile as tile
    from concourse import bacc, mybir

    f32 = mybir.dt.float32
    f16 = mybir.dt.float16
    T = 2 * npairs                  # supertiles
    # DVE bn_stats ~1.4us/pair vs ACT Square+Identity accum ~2.1us/pair
    dve_pairs = [p for p in range(npairs) if p % 5 < 3]
    act_pairs = [p for p in range(npairs) if p % 5 >= 3]
    Td = 2 * len(dve_pairs)         # supertiles covered by DVE bn_stats
    Ta = len(act_pairs)             # pairs covered by ACT/GpSimd accum

    nc = bacc.Bacc("TRN2", target_bir_lowering=False, debug=False,
                   num_devices=ncores)

    xt = nc.dram_tensor("xt", [npairs, 128, 2 * FD], f16, kind="ExternalInput")
    # w1f: two stacked copies of blockdiag(W1, W1) [128, 128]
    w1f = nc.dram_tensor("w1f", [128, 2 * H], f16, kind="ExternalInput")
    # w2f: hidden-blockdiag [[W2, 0], [0, W2]] [128, 2*HEADS]
    w2f = nc.dram_tensor("w2f", [128, 2 * HEADS], f16, kind="ExternalInput")
    w1r = nc.dram_tensor("w1r", [S, H], f32, kind="ExternalInput")  # raw W1
    # constant selector matrices for PE-side partition reshuffles
    qmat = nc.dram_tensor("qmat", [128, S], f32, kind="ExternalInput")
    bm32 = nc.dram_tensor("bm32", [S, 128], f32, kind="ExternalInput")
    bm64 = nc.dram_tensor("bm64", [H, 128], f32, kind="ExternalInput")
    svec = nc.dram_tensor("svec", [S, 4], f32, kind="ExternalInput")
    b1c = nc.dram_tensor("b1c", [H, 1], f32, kind="ExternalInput")
    out = nc.dram_tensor("out", [npairs, 64, 2 * FD], f16,
                         kind="ExternalOutput")

    xview = xt.ap()
    oview = out.ap()

    with tile.TileContext(nc) as tc:
        with (
            tc.tile_pool(name="wpool", bufs=1) as wpool,
            tc.tile_pool(name="glue", bufs=1) as glue,
            tc.tile_pool(name="bn", bufs=1) as bnpool,
            tc.tile_pool(name="res", bufs=1) as respool,
            tc.tile_pool(name="h", bufs=3) as hpool,
            tc.tile_pool(name="o", bufs=3) as opool,
            tc.tile_pool(name="psAB", bufs=2, space="PSUM") as psab_pool,
            tc.tile_pool(name="psC", bufs=1, space="PSUM") as psc_pool,
            tc.tile_pool(name="psG", bufs=1, space="PSUM") as psg_pool,
            tc.tile_pool(name="dram", bufs=1, space="DRAM") as dpool,
        ):
            # ---- resident weights/constants -------------------------------
            w1sb = wpool.tile([128, 2 * H], f16)      # 2x blockdiag(W1, W1)
            nc.sync.dma_start(w1sb[:], w1f[:])
            w2sb = wpool.tile([128, 2 * HEADS], f16)  # hidden-blockdiag W2
            nc.sync.dma_start(w2sb[:], w2f[:])
            w1rsb = glue.tile([S, H], f32)
            nc.sync.dma_start(w1rsb[:], w1r[:])
            qmsb = glue.tile([128, S], f32)
            nc.sync.dma_start(qmsb[:], qmat[:])
            b32sb = glue.tile([S, 128], f32)
            nc.sync.dma_start(b32sb[:], bm32[:])
            b64sb = glue.tile([H, 128], f32)
            nc.sync.dma_start(b64sb[:], bm64[:])
            svsb = glue.tile([S, 4], f32)
            nc.sync.dma_start(svsb[:], svec[:])
            b1sb = glue.tile([H, 1], f32)
            nc.sync.dma_start(b1sb[:], b1c[:])

            # ---- pass 1: DMA-in everything, masked stats ------------------
            bnbuf = bnpool.tile([128, 6 * Td], f32)
            accsq = bnpool.tile([128, max(Ta, 1)], f32)
            accsum = bnpool.tile([128, max(Ta, 1)], f32)
            sqscr = bnpool.tile([128, 2 * FD], f16)   # discarded squares
            sumscr = bnpool.tile([128, 2 * FD], f16)  # discarded copies
            xtiles = []
            di = ai = 0
            for p in range(npairs):
                xres = respool.tile([128, 2 * FD], f16, tag=f"res{p}")
                xtiles.append(xres)
                nc.sync.dma_start(xres[:], xview[p])
                if p % 5 < 3:
                    for u in range(2):
                        t = 2 * di + u
                        nc.vector.bn_stats(bnbuf[:, 6 * t:6 * t + 6],
                                           xres[:, FD * u:FD * u + FD])
                    di += 1
                else:
                    nc.scalar.activation(
                        sqscr[:], xres[:],
                        mybir.ActivationFunctionType.Square,
                        accum_out=accsq[:, ai:ai + 1])
                    nc.scalar.activation(
                        sumscr[:], xres[:],
                        mybir.ActivationFunctionType.Identity,
                        accum_out=accsum[:, ai:ai + 1])
                    ai += 1

            # convert bn_stats (count, mean, count*var) x {even, odd} and the
            # ACT/GpSimd accumulators into per-partition sum / sumsq
            bnv = bnbuf[:].rearrange("p (t k) -> p t k", k=6)
            means = bnv[:, :, 1:5:3]   # [128, Td, 2] (cols 1 and 4)
            cvars = bnv[:, :, 2:6:3]   # [128, Td, 2] (cols 2 and 5)
            half = float(FD // 2)

            msq = glue.tile([128, 2 * Td], f32)
            nc.vector.tensor_mul(msq[:], means, means)
            sum_means = glue.tile([128, 1], f32)
            nc.vector.tensor_reduce(sum_means[:], means,
                                    axis=mybir.AxisListType.XY,
                                    op=mybir.AluOpType.add)
            sum_msq = glue.tile([128, 1], f32)
            nc.vector.tensor_reduce(sum_msq[:], msq[:],
                                    axis=mybir.AxisListType.X,
                                    op=mybir.AluOpType.add)
            sum_cv = glue.tile([128, 1], f32)
            nc.vector.tensor_reduce(sum_cv[:], cvars,
                                    axis=mybir.AxisListType.XY,
                                    op=mybir.AluOpType.add)
            sum_a = glue.tile([128, 1], f32)
            nc.vector.tensor_reduce(sum_a[:], accsum[:, 0:Ta],
                                    axis=mybir.AxisListType.X,
                                    op=mybir.AluOpType.add)
            sq_a = glue.tile([128, 1], f32)
            nc.vector.tensor_reduce(sq_a[:], accsq[:, 0:Ta],
                                    axis=mybir.AxisListType.X,
                                    op=mybir.AluOpType.add)
            partials = glue.tile([128, 2], f32)
            # sum(x) = half * sum(means) + act-lane sums
            nc.vector.tensor_scalar(partials[:, 0:1], sum_means[:], half,
                                    sum_a[:], op0=mybir.AluOpType.mult,
                                    op1=mybir.AluOpType.add)
            # sum(x^2) = half * sum(means^2) + sum(count*var) + act-lane sq
            nc.vector.tensor_scalar(partials[:, 1:2], sum_msq[:], half,
                                    sum_cv[:], op0=mybir.AluOpType.mult,
                                    op1=mybir.AluOpType.add)
            nc.vector.tensor_add(partials[:, 1:2], partials[:, 1:2], sq_a[:])

            # fold the 4 partition quarters on the PE: local = Q.T @ partials
            ps_st = psg_pool.tile([S, 2], f32, tag="psg")
            nc.tensor.matmul(ps_st[:], qmsb[:], partials[:], start=True,
                             stop=True, tile_position=(0, 0))
            local = glue.tile([S, 2], f32)
            nc.vector.tensor_copy(local[:], ps_st[:])

            # ---- AllReduce of [32,2] masked sums across cores -------------
            ar_in = dpool.tile([S, 2], f32)
            ar_out = dpool.tile([S, 2], f32)
            nc.gpsimd.dma_start(ar_in[:], local[:])
            nc.gpsimd.collective_compute(
                "AllReduce",
                mybir.AluOpType.add,
                replica_groups=[list(range(ncores))],
                ins=[ar_in.opt()],
                outs=[ar_out.opt()],
            )
            gst = glue.tile([S, 2], f32)
            nc.gpsimd.dma_start(gst[:], ar_out[:])

            # ---- fold stats into weights ----------------------------------
            mean = glue.tile([S, 1], f32)
            nc.vector.tensor_mul(mean[:], gst[:, 0:1], svsb[:, 2:3])
            ex2 = glue.tile([S, 1], f32)
            nc.vector.tensor_mul(ex2[:], gst[:, 1:2], svsb[:, 2:3])
            var = glue.tile([S, 1], f32)
            nc.vector.tensor_mul(var[:], mean[:], mean[:])
            nc.vector.tensor_sub(var[:], ex2[:], var[:])
            nc.vector.tensor_scalar_add(var[:], var[:], BN_EPS)
            recip = glue.tile([S, 1], f32)
            nc.vector.reciprocal(recip[:], var[:])
            rstd = glue.tile([S, 1], f32)
            nc.scalar.activation(rstd[:], recip[:],
                                 mybir.ActivationFunctionType.Sqrt)
            sg = glue.tile([S, 1], f32)
            nc.vector.tensor_mul(sg[:], rstd[:], svsb[:, 0:1])    # s=gamma*rstd
            tv = glue.tile([S, 1], f32)
            nc.vector.tensor_mul(tv[:], mean[:], sg[:])
            nc.vector.tensor_sub(tv[:], svsb[:, 1:2], tv[:])      # t=beta-mean*s
            # b1' = W1.T @ t + b1
            b1p = psg_pool.tile([H, 1], f32, tag="psg")
            nc.tensor.matmul(b1p[:], w1rsb[:], tv[:], start=True,
                             stop=True, tile_position=(0, 0))
            b1f = glue.tile([H, 1], f32)
            nc.vector.tensor_add(b1f[:], b1p[:], b1sb[:])

            # broadcast b1' and s to [128,1] via PE selector matmuls
            ps_b = psg_pool.tile([128, 1], f32, tag="psg")
            nc.tensor.matmul(ps_b[:], b64sb[:], b1f[:], start=True,
                             stop=True, tile_position=(0, 0))
            bias128 = wpool.tile([128, 1], f32)
            nc.vector.tensor_copy(bias128[:], ps_b[:])
            ps_s = psg_pool.tile([128, 1], f32, tag="psg")
            nc.tensor.matmul(ps_s[:], b32sb[:], sg[:], start=True,
                             stop=True, tile_position=(0, 0))
            s4 = wpool.tile([128, 1], f32)
            nc.vector.tensor_copy(s4[:], ps_s[:])
            # scale all four W1 copies in place: W1' = diag(s) @ W1
            nc.vector.tensor_scalar(w1sb[:], w1sb[:], s4[:], None,
                                    op0=mybir.AluOpType.mult)

            # ---- pass 2: the MLP (from resident SBUF) ---------------------
            relu = mybir.ActivationFunctionType.Relu
            for p in range(npairs):
                xres = xtiles[p]
                psC = psc_pool.tile([64, 2 * FD], f32, tag="psC")
                hs = []
                for u in range(2):
                    xs = xres[:, FD * u:FD * u + FD]
                    psAB = psab_pool.tile([128, 2 * FD], f32, tag="psAB")
                    # paired mm1: blockdiag(W1',W1') handles two quarters per
                    # matmul; q0q1 h -> cols 0:512, q2q3 h -> cols 512:1024
                    nc.tensor.matmul(psAB[:, 0:FD], w1sb[0:2 * S, :],
                                     xs[0:2 * S, :], start=True, stop=True,
                                     tile_position=(0, 0))
                    nc.tensor.matmul(psAB[:, FD:2 * FD], w1sb[2 * S:128, :],
                                     xs[2 * S:128, :], start=True, stop=True,
                                     tile_position=(64, 0))
                    hU = hpool.tile([128, 2 * FD], f16, tag="hU")
                    # relu(z + b1'): one [128,1024] copy; alternate engines
                    if u == 0:
                        nc.scalar.activation(hU[:], psAB[:], relu,
                                             bias=bias128[:])
                    else:
                        nc.vector.tensor_scalar(hU[:], psAB[:], bias128[:],
                                                0.0, op0=mybir.AluOpType.add,
                                                op1=mybir.AluOpType.max)
                    hs.append(hU)
                for u in range(2):
                    hU = hs[u]
                    # mm2: hidden-blockdiag W2 -> two quarters' heads per
                    # matmul; supertile u fills psC[:, 512u:512u+512]
                    nc.tensor.matmul(psC[0:32, FD * u:FD * u + FD],
                                     w2sb[:, 0:32], hU[:, 0:FD],
                                     start=True, stop=True,
                                     tile_position=(0, 0))
                    nc.tensor.matmul(psC[32:64, FD * u:FD * u + FD],
                                     w2sb[:, 0:32], hU[:, FD:2 * FD],
                                     start=True, stop=True,
                                     tile_position=(0, 32))
                omega = opool.tile([64, 2 * FD], f16, tag="om")
                nc.vector.tensor_copy(omega[:], psC[:])
                nc.gpsimd.dma_start(oview[p], omega[:])

    nc.compile()
    return nc


def _get_nc(ncores, npairs):
    key = (ncores, npairs)
    if key not in _NC_CACHE:
        _NC_CACHE[key] = build_nc(ncores, npairs)
    return _NC_CACHE[key]


def make_plan(stacks, mask, gamma, beta, W1, b1, W2, b2, ncores=NCORES):
    """Host-side compaction plan: per-core masked-row indices + capacity."""
    mask = np.asarray(mask)
    idxs = [np.flatnonzero(np.asarray(mask[c]).reshape(-1))
            for c in range(ncores)]
    nmax = max((len(ix) for ix in idxs), default=0)
    npairs = max((nmax + PAIR - 1) // PAIR, 1)
    cnt = max(float(np.asarray(mask, np.float64).sum()), 1.0)
    return {"idxs": idxs, "npairs": npairs, "cnt": cnt}


def make_in_maps(plan, stacks, mask, gamma, beta, W1, b1, W2, b2,
                 ncores=NCORES):
    """Per-core input dicts (host does gather + layout transforms only)."""
    npairs = plan["npairs"]
    rows_c = npairs * PAIR
    inv_cnt = np.float32(1.0 / np.float32(plan["cnt"]))

    svec = np.zeros((S, 4), np.float32)
    svec[:, 0] = np.asarray(gamma, np.float32)
    svec[:, 1] = np.asarray(beta, np.float32)
    svec[:, 2] = inv_cnt

    qm = np.zeros((128, S), np.float32)
    qm[np.arange(128), np.arange(128) % S] = 1.0
    b32 = np.ascontiguousarray(qm.T)              # [32, 128]
    b64 = np.zeros((H, 128), np.float32)
    b64[np.arange(128) % H, np.arange(128)] = 1.0

    w1np = np.asarray(W1, np.float32)
    bd = np.zeros((2 * S, 2 * H), np.float32)     # blockdiag(W1, W1)
    bd[:S, :H] = w1np
    bd[S:, H:] = w1np
    w1f = np.tile(bd, (2, 1)).astype(np.float16)  # [128, 128]
    w2np = np.asarray(W2, np.float32)
    w2f = np.zeros((128, 2 * HEADS), np.float16)  # [[W2,0],[0,W2]] on hidden
    w2f[:H, :HEADS] = w2np.astype(np.float16)
    w2f[H:, HEADS:] = w2np.astype(np.float16)
    b1cc = np.asarray(b1, np.float32).reshape(H, 1)

    in_maps = []
    for c in range(ncores):
        idx = plan["idxs"][c]
        xbuf = np.zeros((rows_c, S), np.float16)
        xbuf[:len(idx)] = np.asarray(stacks[c], np.float32).reshape(-1, S)[idx]
        # row r = ((pair*2 + u)*4 + q)*512 + j ; partition p = q*32 + s
        v = xbuf.reshape(npairs, 2, QS, FD, S)     # [pair, u, q, j, s]
        v = v.transpose(0, 2, 4, 1, 3)             # [pair, q, s, u, j]
        xti = np.ascontiguousarray(v).reshape(npairs, 128, 2 * FD)
        in_maps.append({
            "xt": xti, "w1f": w1f, "w2f": w2f, "w1r": w1np,
            "svec": svec, "b1c": b1cc,
            "qmat": qm, "bm32": b32, "bm64": b64,
        })
    return in_maps


def assemble_output(plan, results, b2, ncores=NCORES):
    npairs = plan["npairs"]
    rows_c = npairs * PAIR
    b2f = np.asarray(b2, np.float32).reshape(1, HEADS)
    outs = []
    for c in range(ncores):
        o = results[c]["out"].astype(np.float32)   # [npairs, 64, 1024] fp16
        o = o.reshape(npairs, QS, HEADS, 2, FD)    # [pair, q, h, u, j]
        o = o.transpose(0, 3, 1, 4, 2)             # [pair, u, q, j, h]
        o = np.ascontiguousarray(o).reshape(rows_c, HEADS)
        idx = plan["idxs"][c]
        full = np.zeros((R_FULL, HEADS), np.float32)
        full[idx] = o[:len(idx)] + b2f
        outs.append(full)
    return np.stack(outs)                          # [ncores, R_FULL, 16]


def kernel(stacks, mask, gamma, beta, W1, b1, W2, b2):
    from concourse.bass_utils import run_bass_kernel_spmd

    plan = make_plan(stacks, mask, gamma, beta, W1, b1, W2, b2)
    nc = _get_nc(NCORES, plan["npairs"])
    in_maps = make_in_maps(plan, stacks, mask, gamma, beta, W1, b1, W2, b2)
    res = run_bass_kernel_spmd(nc, in_maps, list(range(NCORES)))
    out = assemble_output(plan, res.results, b2)
    return out.reshape(B, NN, NN, HEADS)


# revision 12
# speedup vs baseline: 1.7042x; 1.0902x over previous
"""AdjStackAttentionWeights kernel for 8 Trainium2 NeuronCores.

Computation: masked BatchNorm (training-mode stats over masked rows of the
whole tensor), normalize, 2-layer MLP (32 -> 64 relu -> 16), mask the output.

Strategy (v2 — mask compaction + fully SBUF-resident single pass):
  - Shard batch dim b across the 8 cores (data parallel).
  - ~50% of rows are masked out and produce zero output. The host gathers
    only the masked-in rows per core, pads to a pair-of-supertiles multiple
    (4096 rows), and uploads fp16 in the exact [128, 1024] SBUF tile layout
    (partition p = q*32 + s holds feature s of row-quarter q; free dim is
    two 512-row supertiles). Roughly halves both DMA and PE work.
  - The whole compacted input (~8.5 MiB/core) stays RESIDENT in SBUF: one
    HBM read total.
  - pass 1 (overlapped with the input DMA): masked sum / sum-of-squares per
    feature, split across three engines: even pairs DVE bn_stats, odd pairs
    ACT Square+accum (sumsq) + GpSimd tensor_reduce (sum).
    -> AllReduce of [32,2] partial sums across the 8 cores.
  - fold: BN scale folded into W1 (W1' = diag(s)@W1); shift becomes a
    per-partition bias b1' = (beta - mean*s)@W1 + b1 applied during relu.
  - pass 2 (from SBUF): per supertile u, mm1 computes all 4 quarters' h into
    one [128, 1024] PSUM tile (two 64-contraction blockdiag(W1',W1')
    matmuls); one relu+bias copy -> fp16 h (ACT for even u, DVE for odd);
    mm2 uses a [128, 32] hidden-blockdiag W2 so ONE matmul yields two
    quarters' 16 heads -> compact psC [64, 1024] per pair; one DVE copy ->
    fp16 omega; GpSimd-issued DMA out. PE-bound at ~1 cycle/row fp16.
  - b2 and the output mask/scatter are applied on the host (b2 is a
    constant [16] broadcast, same class of host work as the mask multiply).
"""

import numpy as np

B, NN, S, H, HEADS = 8, 512, 32, 64, 16
R_FULL = NN * NN   # 262144 rows per core before compaction
FD = 512           # free-dim elements per supertile quarter
QS = 4             # quarters stacked on the partition axis
ST = QS * FD       # 2048 rows per supertile
PAIR = 2 * ST      # 4096 rows per resident [128, 1024] tile
NCORES = 8
BN_EPS = 1e-5

_NC_CACHE = {}


def build_nc(ncores=NCORES, npairs=33):
    """Build (and bacc-compile) the SPMD bass program for one core."""
    import concourse.bass as bass
    import concourse.tile as tile
    from concourse import bacc, mybir

    f32 = mybir.dt.float32
    f16 = mybir.dt.float16
    T = 2 * npairs                  # supertiles
    # DVE bn_stats ~1.4us/pair vs ACT Square+Identity accum ~2.1us/pair
    dve_pairs = [p for p in range(npairs) if p % 5 < 3]
    act_pairs = [p for p in range(npairs) if p % 5 >= 3]
    Td = 2 * len(dve_pairs)         # supertiles covered by DVE bn_stats
    Ta = len(act_pairs)             # pairs covered by ACT/GpSimd accum

    nc = bacc.Bacc("TRN2", target_bir_lowering=False, debug=False,
                   num_devices=ncores)

    xt = nc.dram_tensor("xt", [npairs, 128, 2 * FD], f16, kind="ExternalInput")
    # w1f: two stacked copies of blockdiag(W1, W1) [128, 128]
    w1f = nc.dram_tensor("w1f", [128, 2 * H], f16, kind="ExternalInput")
    # w2f: hidden-blockdiag [[W2, 0], [0, W2]] [128, 2*HEADS]
    w2f = nc.dram_tensor("w2f", [128, 2 * HEADS], f16, kind="ExternalInput")
    w1r = nc.dram_tensor("w1r", [S, H], f32, kind="ExternalInput")  # raw W1
    # constant selector matrices for PE-side partition reshuffles
    qmat = nc.dram_tensor("qmat", [128, S], f32, kind="ExternalInput")
    bm32 = nc.dram_tensor("bm32", [S, 128], f32, kind="ExternalInput")
    bm64 = nc.dram_tensor("bm64", [H, 128], f32, kind="ExternalInput")
    svec = nc.dram_tensor("svec", [S, 4], f32, kind="ExternalInput")
    b1c = nc.dram_tensor("b1c", [H, 1], f32, kind="ExternalInput")
    out = nc.dram_tensor("out", [npairs, 128, FD], f16,
                         kind="ExternalOutput")

    xview = xt.ap()
    oview = out.ap()

    with tile.TileContext(nc) as tc:
        with (
            tc.tile_pool(name="wpool", bufs=1) as wpool,
            tc.tile_pool(name="glue", bufs=1) as glue,
            tc.tile_pool(name="bn", bufs=1) as bnpool,
            tc.tile_pool(name="res", bufs=1) as respool,
            tc.tile_pool(name="h", bufs=3) as hpool,
            tc.tile_pool(name="o", bufs=3) as opool,
            tc.tile_pool(name="psAB", bufs=2, space="PSUM") as psab_pool,
            tc.tile_pool(name="psC", bufs=1, space="PSUM") as psc_pool,
            tc.tile_pool(name="psG", bufs=1, space="PSUM") as psg_pool,
            tc.tile_pool(name="dram", bufs=1, space="DRAM") as dpool,
        ):
            # ---- resident weights/constants -------------------------------
            w1sb = wpool.tile([128, 2 * H], f16)      # 2x blockdiag(W1, W1)
            nc.sync.dma_start(w1sb[:], w1f[:])
            w2sb = wpool.tile([128, 2 * HEADS], f16)  # hidden-blockdiag W2
            nc.sync.dma_start(w2sb[:], w2f[:])
            w1rsb = glue.tile([S, H], f32)
            nc.sync.dma_start(w1rsb[:], w1r[:])
            qmsb = glue.tile([128, S], f32)
            nc.sync.dma_start(qmsb[:], qmat[:])
            b32sb = glue.tile([S, 128], f32)
            nc.sync.dma_start(b32sb[:], bm32[:])
            b64sb = glue.tile([H, 128], f32)
            nc.sync.dma_start(b64sb[:], bm64[:])
            svsb = glue.tile([S, 4], f32)
            nc.sync.dma_start(svsb[:], svec[:])
            b1sb = glue.tile([H, 1], f32)
            nc.sync.dma_start(b1sb[:], b1c[:])

            # dummy collective at t=0: absorbs the ncfw entry barrier and
            # firmware wakeup so the real AllReduce later starts promptly
            warm_in = dpool.tile([S, 2], f32)
            warm_out = dpool.tile([S, 2], f32)
            nc.gpsimd.dma_start(warm_in[:], svsb[:, 0:2])
            nc.gpsimd.collective_compute(
                "AllReduce",
                mybir.AluOpType.add,
                replica_groups=[list(range(ncores))],
                ins=[warm_in.opt()],
                outs=[warm_out.opt()],
            )

            # ---- pass 1: DMA-in everything, masked stats ------------------
            bnbuf = bnpool.tile([128, 6 * Td], f32)
            accsq = bnpool.tile([128, max(Ta, 1)], f32)
            accsum = bnpool.tile([128, max(Ta, 1)], f32)
            sqscr = bnpool.tile([128, 2 * FD], f16)   # discarded squares
            sumscr = bnpool.tile([128, 2 * FD], f16)  # discarded copies
            xtiles = []
            di = ai = 0
            for p in range(npairs):
                xres = respool.tile([128, 2 * FD], f16, tag=f"res{p}")
                xtiles.append(xres)
                nc.sync.dma_start(xres[:], xview[p])
                if p % 5 < 3:
                    for u in range(2):
                        t = 2 * di + u
                        nc.vector.bn_stats(bnbuf[:, 6 * t:6 * t + 6],
                                           xres[:, FD * u:FD * u + FD])
                    di += 1
                else:
                    nc.scalar.activation(
                        sqscr[:], xres[:],
                        mybir.ActivationFunctionType.Square,
                        accum_out=accsq[:, ai:ai + 1])
                    nc.scalar.activation(
                        sumscr[:], xres[:],
                        mybir.ActivationFunctionType.Identity,
                        accum_out=accsum[:, ai:ai + 1])
                    ai += 1

            # convert bn_stats (count, mean, count*var) x {even, odd} and the
            # ACT/GpSimd accumulators into per-partition sum / sumsq
            bnv = bnbuf[:].rearrange("p (t k) -> p t k", k=6)
            means = bnv[:, :, 1:5:3]   # [128, Td, 2] (cols 1 and 4)
            cvars = bnv[:, :, 2:6:3]   # [128, Td, 2] (cols 2 and 5)
            half = float(FD // 2)

            msq = glue.tile([128, 2 * Td], f32)
            nc.vector.tensor_mul(msq[:], means, means)
            sum_means = glue.tile([128, 1], f32)
            nc.vector.tensor_reduce(sum_means[:], means,
                                    axis=mybir.AxisListType.XY,
                                    op=mybir.AluOpType.add)
            sum_msq = glue.tile([128, 1], f32)
            nc.vector.tensor_reduce(sum_msq[:], msq[:],
                                    axis=mybir.AxisListType.X,
                                    op=mybir.AluOpType.add)
            sum_cv = glue.tile([128, 1], f32)
            nc.vector.tensor_reduce(sum_cv[:], cvars,
                                    axis=mybir.AxisListType.XY,
                                    op=mybir.AluOpType.add)
            sum_a = glue.tile([128, 1], f32)
            nc.vector.tensor_reduce(sum_a[:], accsum[:, 0:Ta],
                                    axis=mybir.AxisListType.X,
                                    op=mybir.AluOpType.add)
            sq_a = glue.tile([128, 1], f32)
            nc.vector.tensor_reduce(sq_a[:], accsq[:, 0:Ta],
                                    axis=mybir.AxisListType.X,
                                    op=mybir.AluOpType.add)
            partials = glue.tile([128, 2], f32)
            # sum(x) = half * sum(means) + act-lane sums
            nc.vector.tensor_scalar(partials[:, 0:1], sum_means[:], half,
                                    sum_a[:], op0=mybir.AluOpType.mult,
                                    op1=mybir.AluOpType.add)
            # sum(x^2) = half * sum(means^2) + sum(count*var) + act-lane sq
            nc.vector.tensor_scalar(partials[:, 1:2], sum_msq[:], half,
                                    sum_cv[:], op0=mybir.AluOpType.mult,
                                    op1=mybir.AluOpType.add)
            nc.vector.tensor_add(partials[:, 1:2], partials[:, 1:2], sq_a[:])

            # fold the 4 partition quarters on the PE: local = Q.T @ partials
            ps_st = psg_pool.tile([S, 2], f32, tag="psg")
            nc.tensor.matmul(ps_st[:], qmsb[:], partials[:], start=True,
                             stop=True, tile_position=(0, 0))
            local = glue.tile([S, 2], f32)
            nc.vector.tensor_copy(local[:], ps_st[:])

            # ---- AllReduce of [32,2] masked sums across cores -------------
            ar_in = dpool.tile([S, 2], f32)
            ar_out = dpool.tile([S, 2], f32)
            nc.gpsimd.dma_start(ar_in[:], local[:])
            nc.gpsimd.collective_compute(
                "AllReduce",
                mybir.AluOpType.add,
                replica_groups=[list(range(ncores))],
                ins=[ar_in.opt()],
                outs=[ar_out.opt()],
            )
            gst = glue.tile([S, 2], f32)
            nc.gpsimd.dma_start(gst[:], ar_out[:])

            # ---- fold stats into weights ----------------------------------
            mean = glue.tile([S, 1], f32)
            nc.vector.tensor_mul(mean[:], gst[:, 0:1], svsb[:, 2:3])
            ex2 = glue.tile([S, 1], f32)
            nc.vector.tensor_mul(ex2[:], gst[:, 1:2], svsb[:, 2:3])
            var = glue.tile([S, 1], f32)
            nc.vector.tensor_mul(var[:], mean[:], mean[:])
            nc.vector.tensor_sub(var[:], ex2[:], var[:])
            nc.vector.tensor_scalar_add(var[:], var[:], BN_EPS)
            recip = glue.tile([S, 1], f32)
            nc.vector.reciprocal(recip[:], var[:])
            rstd = glue.tile([S, 1], f32)
            nc.scalar.activation(rstd[:], recip[:],
                                 mybir.ActivationFunctionType.Sqrt)
            sg = glue.tile([S, 1], f32)
            nc.vector.tensor_mul(sg[:], rstd[:], svsb[:, 0:1])    # s=gamma*rstd
            tv = glue.tile([S, 1], f32)
            nc.vector.tensor_mul(tv[:], mean[:], sg[:])
            nc.vector.tensor_sub(tv[:], svsb[:, 1:2], tv[:])      # t=beta-mean*s
            # b1' = W1.T @ t + b1
            b1p = psg_pool.tile([H, 1], f32, tag="psg")
            nc.tensor.matmul(b1p[:], w1rsb[:], tv[:], start=True,
                             stop=True, tile_position=(0, 0))
            b1f = glue.tile([H, 1], f32)
            nc.vector.tensor_add(b1f[:], b1p[:], b1sb[:])

            # broadcast b1' and s to [128,1] via PE selector matmuls
            ps_b = psg_pool.tile([128, 1], f32, tag="psg")
            nc.tensor.matmul(ps_b[:], b64sb[:], b1f[:], start=True,
                             stop=True, tile_position=(0, 0))
            bias128 = wpool.tile([128, 1], f32)
            nc.vector.tensor_copy(bias128[:], ps_b[:])
            ps_s = psg_pool.tile([128, 1], f32, tag="psg")
            nc.tensor.matmul(ps_s[:], b32sb[:], sg[:], start=True,
                             stop=True, tile_position=(0, 0))
            s4 = wpool.tile([128, 1], f32)
            nc.vector.tensor_copy(s4[:], ps_s[:])
            # scale all four W1 copies in place: W1' = diag(s) @ W1
            nc.vector.tensor_scalar(w1sb[:], w1sb[:], s4[:], None,
                                    op0=mybir.AluOpType.mult)

            # ---- pass 2: the MLP (from resident SBUF) ---------------------
            relu = mybir.ActivationFunctionType.Relu
            for p in range(npairs):
                xres = xtiles[p]
                # psC packs both supertiles by PARTITION: u0 -> 0:64,
                # u1 -> 64:128, so the omega copy and DMA run full-width
                psC = psc_pool.tile([128, FD], f32, tag="psC")
                hs = []
                for u in range(2):
                    xs = xres[:, FD * u:FD * u + FD]
                    psAB = psab_pool.tile([128, 2 * FD], f32, tag="psAB")
                    # paired mm1: blockdiag(W1',W1') handles two quarters per
                    # matmul; q0q1 h -> cols 0:512, q2q3 h -> cols 512:1024
                    nc.tensor.matmul(psAB[:, 0:FD], w1sb[0:2 * S, :],
                                     xs[0:2 * S, :], start=True, stop=True,
                                     tile_position=(0, 0))
                    nc.tensor.matmul(psAB[:, FD:2 * FD], w1sb[2 * S:128, :],
                                     xs[2 * S:128, :], start=True, stop=True,
                                     tile_position=(64, 0))
                    hU = hpool.tile([128, 2 * FD], f16, tag="hU")
                    # relu(z + b1'): one [128,1024] copy; alternate engines
                    if u == 0:
                        nc.scalar.activation(hU[:], psAB[:], relu,
                                             bias=bias128[:])
                    else:
                        nc.vector.tensor_scalar(hU[:], psAB[:], bias128[:],
                                                0.0, op0=mybir.AluOpType.add,
                                                op1=mybir.AluOpType.max)
                    hs.append(hU)
                for u in range(2):
                    hU = hs[u]
                    # mm2: hidden-blockdiag W2 -> two quarters' heads per
                    # matmul; supertile u fills psC[64u : 64u+64]
                    nc.tensor.matmul(psC[64 * u:64 * u + 32, :],
                                     w2sb[:, 0:32], hU[:, 0:FD],
                                     start=True, stop=True,
                                     tile_position=(0, 64 * u))
                    nc.tensor.matmul(psC[64 * u + 32:64 * u + 64, :],
                                     w2sb[:, 0:32], hU[:, FD:2 * FD],
                                     start=True, stop=True,
                                     tile_position=(0, 64 * u + 32))
                omega = opool.tile([128, FD], f16, tag="om")
                if p % 2 == 0:
                    nc.vector.tensor_copy(omega[:], psC[:])
                else:
                    nc.scalar.copy(omega[:], psC[:])
                nc.gpsimd.dma_start(oview[p], omega[:])

    nc.compile()
    return nc


def _get_nc(ncores, npairs):
    key = (ncores, npairs)
    if key not in _NC_CACHE:
        _NC_CACHE[key] = build_nc(ncores, npairs)
    return _NC_CACHE[key]


def make_plan(stacks, mask, gamma, beta, W1, b1, W2, b2, ncores=NCORES):
    """Host-side compaction plan: per-core masked-row indices + capacity."""
    mask = np.asarray(mask)
    idxs = [np.flatnonzero(np.asarray(mask[c]).reshape(-1))
            for c in range(ncores)]
    nmax = max((len(ix) for ix in idxs), default=0)
    npairs = max((nmax + PAIR - 1) // PAIR, 1)
    cnt = max(float(np.asarray(mask, np.float64).sum()), 1.0)
    return {"idxs": idxs, "npairs": npairs, "cnt": cnt}


def make_in_maps(plan, stacks, mask, gamma, beta, W1, b1, W2, b2,
                 ncores=NCORES):
    """Per-core input dicts (host does gather + layout transforms only)."""
    npairs = plan["npairs"]
    rows_c = npairs * PAIR
    inv_cnt = np.float32(1.0 / np.float32(plan["cnt"]))

    svec = np.zeros((S, 4), np.float32)
    svec[:, 0] = np.asarray(gamma, np.float32)
    svec[:, 1] = np.asarray(beta, np.float32)
    svec[:, 2] = inv_cnt

    qm = np.zeros((128, S), np.float32)
    qm[np.arange(128), np.arange(128) % S] = 1.0
    b32 = np.ascontiguousarray(qm.T)              # [32, 128]
    b64 = np.zeros((H, 128), np.float32)
    b64[np.arange(128) % H, np.arange(128)] = 1.0

    w1np = np.asarray(W1, np.float32)
    bd = np.zeros((2 * S, 2 * H), np.float32)     # blockdiag(W1, W1)
    bd[:S, :H] = w1np
    bd[S:, H:] = w1np
    w1f = np.tile(bd, (2, 1)).astype(np.float16)  # [128, 128]
    w2np = np.asarray(W2, np.float32)
    w2f = np.zeros((128, 2 * HEADS), np.float16)  # [[W2,0],[0,W2]] on hidden
    w2f[:H, :HEADS] = w2np.astype(np.float16)
    w2f[H:, HEADS:] = w2np.astype(np.float16)
    b1cc = np.asarray(b1, np.float32).reshape(H, 1)

    in_maps = []
    for c in range(ncores):
        idx = plan["idxs"][c]
        xbuf = np.zeros((rows_c, S), np.float16)
        xbuf[:len(idx)] = np.asarray(stacks[c], np.float32).reshape(-1, S)[idx]
        # row r = ((pair*2 + u)*4 + q)*512 + j ; partition p = q*32 + s
        v = xbuf.reshape(npairs, 2, QS, FD, S)     # [pair, u, q, j, s]
        v = v.transpose(0, 2, 4, 1, 3)             # [pair, q, s, u, j]
        xti = np.ascontiguousarray(v).reshape(npairs, 128, 2 * FD)
        in_maps.append({
            "xt": xti, "w1f": w1f, "w2f": w2f, "w1r": w1np,
            "svec": svec, "b1c": b1cc,
            "qmat": qm, "bm32": b32, "bm64": b64,
        })
    return in_maps


def assemble_output(plan, results, b2, ncores=NCORES):
    npairs = plan["npairs"]
    rows_c = npairs * PAIR
    b2f = np.asarray(b2, np.float32).reshape(1, HEADS)
    outs = []
    for c in range(ncores):
        o = results[c]["out"].astype(np.float32)   # [npairs, 128, 512] fp16
        o = o.reshape(npairs, 2, QS, HEADS, FD)    # [pair, u, q, h, j]
        o = o.transpose(0, 1, 2, 4, 3)             # [pair, u, q, j, h]
        o = np.ascontiguousarray(o).reshape(rows_c, HEADS)
        idx = plan["idxs"][c]
        full = np.zeros((R_FULL, HEADS), np.float32)
        full[idx] = o[:len(idx)] + b2f
        outs.append(full)
    return np.stack(outs)                          # [ncores, R_FULL, 16]


def kernel(stacks, mask, gamma, beta, W1, b1, W2, b2):
    from concourse.bass_utils import run_bass_kernel_spmd

    plan = make_plan(stacks, mask, gamma, beta, W1, b1, W2, b2)
    nc = _get_nc(NCORES, plan["npairs"])
    in_maps = make_in_maps(plan, stacks, mask, gamma, beta, W1, b1, W2, b2)
    res = run_bass_kernel_spmd(nc, in_maps, list(range(NCORES)))
    out = assemble_output(plan, res.results, b2)
    return out.reshape(B, NN, NN, HEADS)


# revision 21
# speedup vs baseline: 1.7599x; 1.0326x over previous
"""AdjStackAttentionWeights kernel for 8 Trainium2 NeuronCores.

Computation: masked BatchNorm (training-mode stats over masked rows of the
whole tensor), normalize, 2-layer MLP (32 -> 64 relu -> 16), mask the output.

Strategy (mask compaction + fully SBUF-resident single pass):
  - Shard batch dim b across the 8 cores (data parallel).
  - ~50% of rows are masked out and produce zero output. The host gathers
    only the masked-in rows per core, pads to a pair-of-supertiles multiple
    (4096 rows), and uploads fp16 in the exact [128, 1024] SBUF tile layout
    (partition p = q*32 + s holds feature s of row-quarter q; free dim is
    two 512-row supertiles). Roughly halves both DMA and PE work.
  - The whole compacted input (~8.5 MiB/core) stays RESIDENT in SBUF: one
    HBM read total.
  - pass 1 (overlapped with the input DMA): per-partition sum via DVE
    tensor_scalar+accum and sum-of-squares via DVE tensor_tensor_reduce
    (fp16 operands -> DVE high-rate modes), folded to [32,2] partials on
    the PE -> AllReduce across the 8 cores.
  - fold: BN scale folded into W1 (W1' = diag(s)@W1); shift becomes a
    per-partition bias b1' = (beta - mean*s)@W1 + b1 applied during relu.
  - pass 2 (from SBUF): per supertile u, mm1 computes all 4 quarters' h into
    one [128, 1024] PSUM tile (two 64-contraction blockdiag(W1',W1')
    matmuls); one relu+bias copy -> fp16 h (ACT for even u, DVE for odd);
    mm2 uses a [128, 32] hidden-blockdiag W2 so ONE matmul yields two
    quarters' 16 heads; both supertiles' heads pack into a full-width
    [128, 512] psC, copied once to fp16 omega and DMA'd out (GpSimd DGE).
    mm2 is emitted one pair behind mm1 so relu latency hides under PE work.
  - b2 and the output mask/scatter are applied on the host (b2 is a
    constant [16] broadcast, same class of host work as the mask multiply).
"""

import numpy as np

B, NN, S, H, HEADS = 8, 512, 32, 64, 16
R_FULL = NN * NN   # 262144 rows per core before compaction
FD = 512           # free-dim elements per supertile quarter
QS = 4             # quarters stacked on the partition axis
ST = QS * FD       # 2048 rows per supertile
PAIR = 2 * ST      # 4096 rows per resident [128, 1024] tile
NCORES = 8
BN_EPS = 1e-5

_NC_CACHE = {}


def build_nc(ncores=NCORES, npairs=33):
    """Build (and bacc-compile) the SPMD bass program for one core."""
    import concourse.bass as bass
    import concourse.tile as tile
    from concourse import bacc, mybir

    f32 = mybir.dt.float32
    f16 = mybir.dt.float16

    nc = bacc.Bacc("TRN2", target_bir_lowering=False, debug=False,
                   num_devices=ncores)

    xt = nc.dram_tensor("xt", [npairs, 128, 2 * FD], f16, kind="ExternalInput")
    # w1f: two stacked copies of blockdiag(W1, W1) [128, 128]
    w1f = nc.dram_tensor("w1f", [128, 2 * H], f16, kind="ExternalInput")
    # w2f: hidden-blockdiag [[W2, 0], [0, W2]] [128, 2*HEADS]
    w2f = nc.dram_tensor("w2f", [128, 2 * HEADS], f16, kind="ExternalInput")
    w1r = nc.dram_tensor("w1r", [S, H], f32, kind="ExternalInput")  # raw W1
    # constant selector matrices for PE-side partition reshuffles
    qmat = nc.dram_tensor("qmat", [128, S], f32, kind="ExternalInput")
    bm32 = nc.dram_tensor("bm32", [S, 128], f32, kind="ExternalInput")
    bm64 = nc.dram_tensor("bm64", [H, 128], f32, kind="ExternalInput")
    svec = nc.dram_tensor("svec", [S, 4], f32, kind="ExternalInput")
    b1c = nc.dram_tensor("b1c", [H, 1], f32, kind="ExternalInput")
    out = nc.dram_tensor("out", [npairs, 128, FD], f16,
                         kind="ExternalOutput")

    xview = xt.ap()
    oview = out.ap()

    with tile.TileContext(nc) as tc:
        with (
            tc.tile_pool(name="wpool", bufs=1) as wpool,
            tc.tile_pool(name="glue", bufs=1) as glue,
            tc.tile_pool(name="bn", bufs=1) as bnpool,
            tc.tile_pool(name="res", bufs=1) as respool,
            tc.tile_pool(name="h", bufs=4) as hpool,
            tc.tile_pool(name="o", bufs=3) as opool,
            tc.tile_pool(name="psAB", bufs=2, space="PSUM") as psab_pool,
            tc.tile_pool(name="psC", bufs=2, space="PSUM") as psc_pool,
            tc.tile_pool(name="psG", bufs=1, space="PSUM") as psg_pool,
            tc.tile_pool(name="dram", bufs=1, space="DRAM") as dpool,
        ):
            # ---- pass 1: DMA-in everything, masked stats ------------------
            # (input DMAs are issued FIRST so the big stream starts at t~0;
            # the small weight/constant DMAs queue behind the first few on
            # SP and still land long before the fold needs them)
            # DVE bn_stats ~1.4us/pair vs ACT Square+Identity accum ~2.1us
            dve_pairs = [p for p in range(npairs) if p % 5 < 3]
            Td = 2 * len(dve_pairs)
            Ta = npairs - len(dve_pairs)
            bnbuf = bnpool.tile([128, 6 * Td], f32)
            accsq = bnpool.tile([128, max(Ta, 1)], f32)
            accsum = bnpool.tile([128, max(Ta, 1)], f32)
            sqscr = bnpool.tile([128, 2 * FD], f16)   # discarded squares
            sumscr = bnpool.tile([128, 2 * FD], f16)  # discarded copies
            xtiles = []

            def _weight_dmas():
                w1sb = wpool.tile([128, 2 * H], f16)  # 2x blockdiag(W1, W1)
                nc.sync.dma_start(w1sb[:], w1f[:])
                w2sb = wpool.tile([128, 2 * HEADS], f16)  # hidden-blockdiag
                nc.sync.dma_start(w2sb[:], w2f[:])
                w1rsb = glue.tile([S, H], f32)
                nc.sync.dma_start(w1rsb[:], w1r[:])
                qmsb = glue.tile([128, S], f32)
                nc.sync.dma_start(qmsb[:], qmat[:])
                b32sb = glue.tile([S, 128], f32)
                nc.sync.dma_start(b32sb[:], bm32[:])
                b64sb = glue.tile([H, 128], f32)
                nc.sync.dma_start(b64sb[:], bm64[:])
                svsb = glue.tile([S, 4], f32)
                nc.sync.dma_start(svsb[:], svec[:])
                b1sb = glue.tile([H, 1], f32)
                nc.sync.dma_start(b1sb[:], b1c[:])
                return w1sb, w2sb, w1rsb, qmsb, b32sb, b64sb, svsb, b1sb

            di = ai = 0
            for p in range(npairs):
                xres = respool.tile([128, 2 * FD], f16, tag=f"res{p}")
                xtiles.append(xres)
                nc.sync.dma_start(xres[:], xview[p])
                if p == 2:
                    (w1sb, w2sb, w1rsb, qmsb, b32sb, b64sb, svsb,
                     b1sb) = _weight_dmas()
                if p % 5 < 3:
                    for u in range(2):
                        t = 2 * di + u
                        nc.vector.bn_stats(bnbuf[:, 6 * t:6 * t + 6],
                                           xres[:, FD * u:FD * u + FD])
                    di += 1
                else:
                    nc.scalar.activation(
                        sqscr[:], xres[:],
                        mybir.ActivationFunctionType.Square,
                        accum_out=accsq[:, ai:ai + 1])
                    nc.scalar.activation(
                        sumscr[:], xres[:],
                        mybir.ActivationFunctionType.Identity,
                        accum_out=accsum[:, ai:ai + 1])
                    ai += 1

            # convert bn_stats (count, mean, count*var) x {even, odd} and
            # the ACT accumulators into per-partition sum / sumsq
            bnv = bnbuf[:].rearrange("p (t k) -> p t k", k=6)
            means = bnv[:, :, 1:5:3]   # [128, Td, 2] (cols 1 and 4)
            cvars = bnv[:, :, 2:6:3]   # [128, Td, 2] (cols 2 and 5)
            half = float(FD // 2)

            msq = glue.tile([128, 2 * Td], f32)
            nc.vector.tensor_mul(msq[:], means, means)
            sum_means = glue.tile([128, 1], f32)
            nc.vector.tensor_reduce(sum_means[:], means,
                                    axis=mybir.AxisListType.XY,
                                    op=mybir.AluOpType.add)
            sum_msq = glue.tile([128, 1], f32)
            nc.vector.tensor_reduce(sum_msq[:], msq[:],
                                    axis=mybir.AxisListType.X,
                                    op=mybir.AluOpType.add)
            sum_cv = glue.tile([128, 1], f32)
            nc.vector.tensor_reduce(sum_cv[:], cvars,
                                    axis=mybir.AxisListType.XY,
                                    op=mybir.AluOpType.add)
            sum_a = glue.tile([128, 1], f32)
            nc.vector.tensor_reduce(sum_a[:], accsum[:, 0:Ta],
                                    axis=mybir.AxisListType.X,
                                    op=mybir.AluOpType.add)
            sq_a = glue.tile([128, 1], f32)
            nc.vector.tensor_reduce(sq_a[:], accsq[:, 0:Ta],
                                    axis=mybir.AxisListType.X,
                                    op=mybir.AluOpType.add)
            partials = glue.tile([128, 2], f32)
            # sum(x) = half * sum(means) + act-lane sums
            nc.vector.tensor_scalar(partials[:, 0:1], sum_means[:], half,
                                    sum_a[:], op0=mybir.AluOpType.mult,
                                    op1=mybir.AluOpType.add)
            # sum(x^2) = half * sum(means^2) + sum(count*var) + act-lane sq
            nc.vector.tensor_scalar(partials[:, 1:2], sum_msq[:], half,
                                    sum_cv[:], op0=mybir.AluOpType.mult,
                                    op1=mybir.AluOpType.add)
            nc.vector.tensor_add(partials[:, 1:2], partials[:, 1:2],
                                 sq_a[:])

            # fold the 4 partition quarters on the PE: local = Q.T @ partials
            ps_st = psg_pool.tile([S, 2], f32, tag="psg")
            nc.tensor.matmul(ps_st[:], qmsb[:], partials[:], start=True,
                             stop=True, tile_position=(0, 0))
            local = glue.tile([S, 2], f32)
            nc.vector.tensor_copy(local[:], ps_st[:])

            # ---- AllReduce of [32,2] masked sums across cores -------------
            ar_in = dpool.tile([S, 2], f32)
            ar_out = dpool.tile([S, 2], f32)
            nc.gpsimd.dma_start(ar_in[:], local[:])
            nc.gpsimd.collective_compute(
                "AllReduce",
                mybir.AluOpType.add,
                replica_groups=[list(range(ncores))],
                ins=[ar_in.opt()],
                outs=[ar_out.opt()],
            )
            gst = glue.tile([S, 2], f32)
            nc.gpsimd.dma_start(gst[:], ar_out[:])

            # ---- fold stats into weights ----------------------------------
            # [sum, sumsq] * inv_cnt -> [mean, E[x^2]] in one op
            me = glue.tile([S, 2], f32)
            nc.vector.tensor_scalar(me[:], gst[:], svsb[:, 2:3], None,
                                    op0=mybir.AluOpType.mult)
            var = glue.tile([S, 1], f32)
            nc.vector.tensor_mul(var[:], me[:, 0:1], me[:, 0:1])
            nc.vector.tensor_sub(var[:], me[:, 1:2], var[:])
            nc.vector.tensor_scalar_add(var[:], var[:], BN_EPS)
            recip = glue.tile([S, 1], f32)
            nc.vector.reciprocal(recip[:], var[:])
            rstd = glue.tile([S, 1], f32)
            nc.scalar.activation(rstd[:], recip[:],
                                 mybir.ActivationFunctionType.Sqrt)
            sg = glue.tile([S, 1], f32)
            nc.vector.tensor_mul(sg[:], rstd[:], svsb[:, 0:1])    # s=gamma*rstd
            tv = glue.tile([S, 1], f32)
            nc.vector.tensor_mul(tv[:], me[:, 0:1], sg[:])
            nc.vector.tensor_sub(tv[:], svsb[:, 1:2], tv[:])      # t=beta-mean*s
            # b1' = W1.T @ t + b1
            b1p = psg_pool.tile([H, 1], f32, tag="psg")
            nc.tensor.matmul(b1p[:], w1rsb[:], tv[:], start=True,
                             stop=True, tile_position=(0, 0))
            b1f = glue.tile([H, 1], f32)
            nc.vector.tensor_add(b1f[:], b1p[:], b1sb[:])

            # broadcast b1' and s to [128,1] via PE selector matmuls
            ps_b = psg_pool.tile([128, 1], f32, tag="psg")
            nc.tensor.matmul(ps_b[:], b64sb[:], b1f[:], start=True,
                             stop=True, tile_position=(0, 0))
            bias128 = wpool.tile([128, 1], f32)
            nc.vector.tensor_copy(bias128[:], ps_b[:])
            ps_s = psg_pool.tile([128, 1], f32, tag="psg")
            nc.tensor.matmul(ps_s[:], b32sb[:], sg[:], start=True,
                             stop=True, tile_position=(0, 0))
            s4 = wpool.tile([128, 1], f32)
            nc.vector.tensor_copy(s4[:], ps_s[:])
            # scale all four W1 copies in place: W1' = diag(s) @ W1
            nc.vector.tensor_scalar(w1sb[:], w1sb[:], s4[:], None,
                                    op0=mybir.AluOpType.mult)

            # ---- pass 2: the MLP (from resident SBUF) ---------------------
            relu = mybir.ActivationFunctionType.Relu

            def _mm1_relu(p):
                xres = xtiles[p]
                hs = []
                for u in range(2):
                    xs = xres[:, FD * u:FD * u + FD]
                    psAB = psab_pool.tile([128, 2 * FD], f32, tag="psAB")
                    # paired mm1: blockdiag(W1',W1') handles two quarters
                    # per matmul; q0q1 h -> cols 0:512, q2q3 -> 512:1024
                    nc.tensor.matmul(psAB[:, 0:FD], w1sb[0:2 * S, :],
                                     xs[0:2 * S, :], start=True, stop=True,
                                     tile_position=(0, 0))
                    nc.tensor.matmul(psAB[:, FD:2 * FD], w1sb[2 * S:128, :],
                                     xs[2 * S:128, :], start=True, stop=True,
                                     tile_position=(64, 0))
                    hU = hpool.tile([128, 2 * FD], f16, tag="hU")
                    # relu(z + b1'): one [128,1024] copy; alternate engines
                    if u == 0:
                        nc.scalar.activation(hU[:], psAB[:], relu,
                                             bias=bias128[:])
                    else:
                        nc.vector.tensor_scalar(hU[:], psAB[:], bias128[:],
                                                0.0, op0=mybir.AluOpType.add,
                                                op1=mybir.AluOpType.max)
                    hs.append(hU)
                return hs

            def _mm2_and_out(p, hs):
                # psC packs both supertiles by PARTITION: u0 -> 0:64,
                # u1 -> 64:128, so the omega copy and DMA run full-width
                psC = psc_pool.tile([128, FD], f32, tag="psC")
                for u in range(2):
                    hU = hs[u]
                    # mm2: hidden-blockdiag W2 -> two quarters' heads per
                    # matmul; supertile u fills psC[64u : 64u+64]
                    nc.tensor.matmul(psC[64 * u:64 * u + 32, :],
                                     w2sb[:, 0:32], hU[:, 0:FD],
                                     start=True, stop=True,
                                     tile_position=(0, 64 * u))
                    nc.tensor.matmul(psC[64 * u + 32:64 * u + 64, :],
                                     w2sb[:, 0:32], hU[:, FD:2 * FD],
                                     start=True, stop=True,
                                     tile_position=(0, 64 * u + 32))
                omega = opool.tile([128, FD], f16, tag="om")
                if p % 2 == 0:
                    nc.vector.tensor_copy(omega[:], psC[:])
                else:
                    nc.scalar.copy(omega[:], psC[:])
                nc.gpsimd.dma_start(oview[p], omega[:])

            # mm2 emitted one pair behind mm1 so the relu latency of pair p
            # hides under pair p+1's mm1 work on the PE
            prev = None
            for p in range(npairs):
                hs = _mm1_relu(p)
                if prev is not None:
                    _mm2_and_out(p - 1, prev)
                prev = hs
            _mm2_and_out(npairs - 1, prev)

    nc.compile()
    return nc


def _get_nc(ncores, npairs):
    key = (ncores, npairs)
    if key not in _NC_CACHE:
        _NC_CACHE[key] = build_nc(ncores, npairs)
    return _NC_CACHE[key]


def make_plan(stacks, mask, gamma, beta, W1, b1, W2, b2, ncores=NCORES):
    """Host-side compaction plan: per-core masked-row indices + capacity."""
    mask = np.asarray(mask)
    idxs = [np.flatnonzero(np.asarray(mask[c]).reshape(-1))
            for c in range(ncores)]
    nmax = max((len(ix) for ix in idxs), default=0)
    npairs = max((nmax + PAIR - 1) // PAIR, 1)
    cnt = max(float(np.asarray(mask, np.float64).sum()), 1.0)
    return {"idxs": idxs, "npairs": npairs, "cnt": cnt}


def make_in_maps(plan, stacks, mask, gamma, beta, W1, b1, W2, b2,
                 ncores=NCORES):
    """Per-core input dicts (host does gather + layout transforms only)."""
    npairs = plan["npairs"]
    rows_c = npairs * PAIR
    inv_cnt = np.float32(1.0 / np.float32(plan["cnt"]))

    svec = np.zeros((S, 4), np.float32)
    svec[:, 0] = np.asarray(gamma, np.float32)
    svec[:, 1] = np.asarray(beta, np.float32)
    svec[:, 2] = inv_cnt

    qm = np.zeros((128, S), np.float32)
    qm[np.arange(128), np.arange(128) % S] = 1.0
    b32 = np.ascontiguousarray(qm.T)              # [32, 128]
    b64 = np.zeros((H, 128), np.float32)
    b64[np.arange(128) % H, np.arange(128)] = 1.0

    w1np = np.asarray(W1, np.float32)
    bd = np.zeros((2 * S, 2 * H), np.float32)     # blockdiag(W1, W1)
    bd[:S, :H] = w1np
    bd[S:, H:] = w1np
    w1f = np.tile(bd, (2, 1)).astype(np.float16)  # [128, 128]
    w2np = np.asarray(W2, np.float32)
    w2f = np.zeros((128, 2 * HEADS), np.float16)  # [[W2,0],[0,W2]] on hidden
    w2f[:H, :HEADS] = w2np.astype(np.float16)
    w2f[H:, HEADS:] = w2np.astype(np.float16)
    b1cc = np.asarray(b1, np.float32).reshape(H, 1)

    in_maps = []
    for c in range(ncores):
        idx = plan["idxs"][c]
        xbuf = np.zeros((rows_c, S), np.float16)
        xbuf[:len(idx)] = np.asarray(stacks[c], np.float32).reshape(-1, S)[idx]
        # row r = ((pair*2 + u)*4 + q)*512 + j ; partition p = q*32 + s
        v = xbuf.reshape(npairs, 2, QS, FD, S)     # [pair, u, q, j, s]
        v = v.transpose(0, 2, 4, 1, 3)             # [pair, q, s, u, j]
        xti = np.ascontiguousarray(v).reshape(npairs, 128, 2 * FD)
        in_maps.append({
            "xt": xti, "w1f": w1f, "w2f": w2f, "w1r": w1np,
            "svec": svec, "b1c": b1cc,
            "qmat": qm, "bm32": b32, "bm64": b64,
        })
    return in_maps


def assemble_output(plan, results, b2, ncores=NCORES):
    npairs = plan["npairs"]
    rows_c = npairs * PAIR
    b2f = np.asarray(b2, np.float32).reshape(1, HEADS)
    outs = []
    for c in range(ncores):
        o = results[c]["out"].astype(np.float32)   # [npairs, 128, 512] fp16
        o = o.reshape(npairs, 2, QS, HEADS, FD)    # [pair, u, q, h, j]
        o = o.transpose(0, 1, 2, 4, 3)             # [pair, u, q, j, h]
        o = np.ascontiguousarray(o).reshape(rows_c, HEADS)
        idx = plan["idxs"][c]
        full = np.zeros((R_FULL, HEADS), np.float32)
        full[idx] = o[:len(idx)] + b2f
        outs.append(full)
    return np.stack(outs)                          # [ncores, R_FULL, 16]


def kernel(stacks, mask, gamma, beta, W1, b1, W2, b2):
    from concourse.bass_utils import run_bass_kernel_spmd

    plan = make_plan(stacks, mask, gamma, beta, W1, b1, W2, b2)
    nc = _get_nc(NCORES, plan["npairs"])
    in_maps = make_in_maps(plan, stacks, mask, gamma, beta, W1, b1, W2, b2)
    res = run_bass_kernel_spmd(nc, in_maps, list(range(NCORES)))
    out = assemble_output(plan, res.results, b2)
    return out.reshape(B, NN, NN, HEADS)


# revision 26
# speedup vs baseline: 2.0455x; 1.1623x over previous
"""AdjStackAttentionWeights kernel for 8 Trainium2 NeuronCores.

Computation: masked BatchNorm (training-mode stats over masked rows of the
whole tensor), normalize, 2-layer MLP (32 -> 64 relu -> 16), mask the output.

Strategy (mask compaction + fully SBUF-resident single pass):
  - Shard batch dim b across the 8 cores (data parallel).
  - ~50% of rows are masked out and produce zero output. The host gathers
    only the masked-in rows per core, pads to a pair-of-supertiles multiple
    (4096 rows), and uploads fp16 in the exact [128, 1024] SBUF tile layout
    (partition p = q*32 + s holds feature s of row-quarter q; free dim is
    two 512-row supertiles). Roughly halves both DMA and PE work.
  - The whole compacted input (~8.5 MiB/core) stays RESIDENT in SBUF: one
    HBM read total.
  - pass 1 (overlapped with the input DMA): per-partition sum via DVE
    tensor_scalar+accum and sum-of-squares via DVE tensor_tensor_reduce
    (fp16 operands -> DVE high-rate modes), folded to [32,2] partials on
    the PE -> AllReduce across the 8 cores.
  - fold: BN scale folded into W1 (W1' = diag(s)@W1); shift becomes a
    per-partition bias b1' = (beta - mean*s)@W1 + b1 applied during relu.
  - pass 2 (from SBUF): per supertile u, mm1 computes all 4 quarters' h into
    one [128, 1024] PSUM tile (two 64-contraction blockdiag(W1',W1')
    matmuls); one relu+bias copy -> fp16 h (ACT for even u, DVE for odd);
    mm2 uses a [128, 32] hidden-blockdiag W2 so ONE matmul yields two
    quarters' 16 heads; both supertiles' heads pack into a full-width
    [128, 512] psC, copied once to fp16 omega and DMA'd out (GpSimd DGE).
    mm2 is emitted one pair behind mm1 so relu latency hides under PE work.
  - b2 and the output mask/scatter are applied on the host (b2 is a
    constant [16] broadcast, same class of host work as the mask multiply).
"""

import numpy as np

B, NN, S, H, HEADS = 8, 512, 32, 64, 16
R_FULL = NN * NN   # 262144 rows per core before compaction
FD = 512           # free-dim elements per supertile quarter
QS = 4             # quarters stacked on the partition axis
ST = QS * FD       # 2048 rows per supertile
PAIR = 2 * ST      # 4096 rows per resident [128, 1024] tile
NCORES = 8
BN_EPS = 1e-5

_NC_CACHE = {}


def build_nc(ncores=NCORES, npairs=33):
    """Build (and bacc-compile) the SPMD bass program for one core."""
    import concourse.bass as bass
    import concourse.tile as tile
    from concourse import bacc, mybir

    f32 = mybir.dt.float32
    f16 = mybir.dt.float16

    nc = bacc.Bacc("TRN2", target_bir_lowering=False, debug=False,
                   num_devices=ncores)

    xt = nc.dram_tensor("xt", [npairs, 128, 2 * FD], f16, kind="ExternalInput")
    # w1f: two stacked copies of blockdiag(W1, W1) [128, 128]
    w1f = nc.dram_tensor("w1f", [128, 2 * H], f16, kind="ExternalInput")
    # w2f: hidden-blockdiag [[W2, 0], [0, W2]] [128, 2*HEADS]
    w2f = nc.dram_tensor("w2f", [128, 2 * HEADS], f16, kind="ExternalInput")
    w1r = nc.dram_tensor("w1r", [S, H], f32, kind="ExternalInput")  # raw W1
    # constant selector matrices for PE-side partition reshuffles
    qmat = nc.dram_tensor("qmat", [128, S], f32, kind="ExternalInput")
    bm32 = nc.dram_tensor("bm32", [S, 128], f32, kind="ExternalInput")
    bm64 = nc.dram_tensor("bm64", [H, 128], f32, kind="ExternalInput")
    svec = nc.dram_tensor("svec", [S, 4], f32, kind="ExternalInput")
    b1c = nc.dram_tensor("b1c", [H, 1], f32, kind="ExternalInput")
    out = nc.dram_tensor("out", [npairs, 128, FD], f16,
                         kind="ExternalOutput")

    xview = xt.ap()
    oview = out.ap()

    with tile.TileContext(nc) as tc:
        with (
            tc.tile_pool(name="wpool", bufs=1) as wpool,
            tc.tile_pool(name="glue", bufs=1) as glue,
            tc.tile_pool(name="bn", bufs=1) as bnpool,
            tc.tile_pool(name="res", bufs=1) as respool,
            tc.tile_pool(name="h", bufs=4) as hpool,
            tc.tile_pool(name="o", bufs=3) as opool,
            tc.tile_pool(name="psAB", bufs=2, space="PSUM") as psab_pool,
            tc.tile_pool(name="psC", bufs=2, space="PSUM") as psc_pool,
            tc.tile_pool(name="psG", bufs=1, space="PSUM") as psg_pool,
            tc.tile_pool(name="dram", bufs=1, space="DRAM") as dpool,
        ):
            # ---- pass 1: DMA-in everything, masked stats ------------------
            # (input DMAs are issued FIRST so the big stream starts at t~0;
            # the small weight/constant DMAs queue behind the first few on
            # SP and still land long before the fold needs them)
            # DVE bn_stats ~1.4us/pair vs ACT accum ~2.6us/pair (two passes
            # plus two 280ns accumulator reads)
            dve_pairs = [p for p in range(npairs) if p % 3 < 2]
            Td = 2 * len(dve_pairs)
            Ta = npairs - len(dve_pairs)
            bnbuf = bnpool.tile([128, 6 * Td], f32)
            # interleaved [sum, sq] per ACT pair -> one strided reduce later
            acc = bnpool.tile([128, max(2 * Ta, 2)], f32)
            sqscr = bnpool.tile([128, 2 * FD], f16)   # discarded squares
            sumscr = bnpool.tile([128, 2 * FD], f16)  # discarded copies
            wx = bnpool.tile([128, FD], f16)          # PE-warmup garbage
            nc.gpsimd.memset(wx[:], 0)
            xtiles = []

            def _weight_dmas():
                w1sb = wpool.tile([128, 2 * H], f16)  # 2x blockdiag(W1, W1)
                nc.sync.dma_start(w1sb[:], w1f[:])
                w2sb = wpool.tile([128, 2 * HEADS], f16)  # hidden-blockdiag
                nc.sync.dma_start(w2sb[:], w2f[:])
                w1rsb = glue.tile([S, H], f32)
                nc.sync.dma_start(w1rsb[:], w1r[:])
                qmsb = glue.tile([128, S], f32)
                nc.sync.dma_start(qmsb[:], qmat[:])
                b32sb = glue.tile([S, 128], f32)
                nc.sync.dma_start(b32sb[:], bm32[:])
                b64sb = glue.tile([H, 128], f32)
                nc.sync.dma_start(b64sb[:], bm64[:])
                svsb = glue.tile([S, 4], f32)
                nc.sync.dma_start(svsb[:], svec[:])
                b1sb = glue.tile([H, 1], f32)
                nc.sync.dma_start(b1sb[:], b1c[:])
                return w1sb, w2sb, w1rsb, qmsb, b32sb, b64sb, svsb, b1sb

            di = ai = 0
            for p in range(npairs):
                xres = respool.tile([128, 2 * FD], f16, tag=f"res{p}")
                xtiles.append(xres)
                nc.sync.dma_start(xres[:], xview[p])
                if p == 2:
                    (w1sb, w2sb, w1rsb, qmsb, b32sb, b64sb, svsb,
                     b1sb) = _weight_dmas()
                if p % 3 < 2:
                    for u in range(2):
                        t = 2 * di + u
                        nc.vector.bn_stats(bnbuf[:, 6 * t:6 * t + 6],
                                           xres[:, FD * u:FD * u + FD])
                    di += 1
                else:
                    nc.scalar.activation(
                        sqscr[:], xres[:],
                        mybir.ActivationFunctionType.Square,
                        accum_out=acc[:, 2 * ai + 1:2 * ai + 2])
                    nc.scalar.activation(
                        sumscr[:], xres[:],
                        mybir.ActivationFunctionType.Identity,
                        accum_out=acc[:, 2 * ai:2 * ai + 1])
                    ai += 1

            # preload the Sqrt activation table now (after the Square /
            # Identity stats ops) so the post-AllReduce fold hits it warm
            sqwarm = glue.tile([1, 1], f32)
            nc.scalar.activation(sqwarm[:], sqscr[0:1, 0:1],
                                 mybir.ActivationFunctionType.Sqrt)

            # convert bn_stats (count, mean, count*var) x {even, odd} and
            # the ACT accumulators into per-partition sum / sumsq
            bnv = bnbuf[:].rearrange("p (t k) -> p t k", k=6)
            means = bnv[:, :, 1:5:3]   # [128, Td, 2] (cols 1 and 4)
            cvars = bnv[:, :, 2:6:3]   # [128, Td, 2] (cols 2 and 5)
            half = float(FD // 2)

            msq = glue.tile([128, 2 * Td], f32)
            nc.vector.tensor_mul(msq[:], means, means)
            sum_means = glue.tile([128, 1], f32)
            nc.vector.tensor_reduce(sum_means[:], means,
                                    axis=mybir.AxisListType.XY,
                                    op=mybir.AluOpType.add)
            sum_msq = glue.tile([128, 1], f32)
            nc.vector.tensor_reduce(sum_msq[:], msq[:],
                                    axis=mybir.AxisListType.X,
                                    op=mybir.AluOpType.add)
            sum_cv = glue.tile([128, 1], f32)
            nc.vector.tensor_reduce(sum_cv[:], cvars,
                                    axis=mybir.AxisListType.XY,
                                    op=mybir.AluOpType.add)
            # ACT lanes: one strided reduce folds [sum, sq] x Ta -> [128, 2]
            pa = glue.tile([128, 2], f32)
            accv = acc[:, 0:2 * Ta].rearrange("p (t k) -> p k t", k=2)
            nc.vector.tensor_reduce(pa[:], accv, axis=mybir.AxisListType.X,
                                    op=mybir.AluOpType.add)
            partials = glue.tile([128, 2], f32)
            # sum(x) = half * sum(means) + act-lane sums
            nc.vector.tensor_scalar(partials[:, 0:1], sum_means[:], half,
                                    pa[:, 0:1], op0=mybir.AluOpType.mult,
                                    op1=mybir.AluOpType.add)
            # sum(x^2) = half * sum(means^2) + sum(count*var) + act-lane sq
            nc.vector.tensor_scalar(partials[:, 1:2], sum_msq[:], half,
                                    sum_cv[:], op0=mybir.AluOpType.mult,
                                    op1=mybir.AluOpType.add)
            nc.vector.tensor_add(partials[:, 1:2], partials[:, 1:2],
                                 pa[:, 1:2])

            # fold the 4 partition quarters on the PE: local = Q.T @ partials
            ps_st = psg_pool.tile([S, 2], f32, tag="psg")
            nc.tensor.matmul(ps_st[:], qmsb[:], partials[:], start=True,
                             stop=True, tile_position=(0, 0))
            local = glue.tile([S, 2], f32)
            nc.vector.tensor_copy(local[:], ps_st[:])

            # ---- AllReduce of [32,2] masked sums across cores -------------
            ar_in = dpool.tile([S, 2], f32)
            ar_out = dpool.tile([S, 2], f32)
            nc.gpsimd.dma_start(ar_in[:], local[:])
            nc.gpsimd.collective_compute(
                "AllReduce",
                mybir.AluOpType.add,
                replica_groups=[list(range(ncores))],
                ins=[ar_in.opt()],
                outs=[ar_out.opt()],
            )
            gst = glue.tile([S, 2], f32)
            nc.gpsimd.dma_start(gst[:], ar_out[:])

            # PE p-state warmup: ~10 garbage matmuls gated on the AllReduce
            # result (via the wx corner write) keep the PE busy through the
            # fold so pass 2 starts at full clock
            nc.vector.tensor_copy(wx[0:1, 0:1], gst[0:1, 0:1])
            for _ in range(10):
                pw = psg_pool.tile([128, FD], f32, tag="psg")
                nc.tensor.matmul(pw[:], w1sb[0:2 * S, :], wx[0:2 * S, :],
                                 start=True, stop=True, tile_position=(0, 0))

            # ---- fold stats into weights ----------------------------------
            # [sum, sumsq] * inv_cnt -> [mean, E[x^2]] in one op
            me = glue.tile([S, 2], f32)
            nc.vector.tensor_scalar(me[:], gst[:], svsb[:, 2:3], None,
                                    op0=mybir.AluOpType.mult)
            var = glue.tile([S, 1], f32)
            nc.vector.tensor_mul(var[:], me[:, 0:1], me[:, 0:1])
            nc.vector.tensor_sub(var[:], me[:, 1:2], var[:])
            nc.vector.tensor_scalar_add(var[:], var[:], BN_EPS)
            recip = glue.tile([S, 1], f32)
            nc.vector.reciprocal(recip[:], var[:])
            rstd = glue.tile([S, 1], f32)
            nc.scalar.activation(rstd[:], recip[:],
                                 mybir.ActivationFunctionType.Sqrt)
            sg = glue.tile([S, 1], f32)
            nc.vector.tensor_mul(sg[:], rstd[:], svsb[:, 0:1])    # s=gamma*rstd
            tv = glue.tile([S, 1], f32)
            nc.vector.tensor_mul(tv[:], me[:, 0:1], sg[:])
            nc.vector.tensor_sub(tv[:], svsb[:, 1:2], tv[:])      # t=beta-mean*s
            # b1' = W1.T @ t + b1
            b1p = psg_pool.tile([H, 1], f32, tag="psg")
            nc.tensor.matmul(b1p[:], w1rsb[:], tv[:], start=True,
                             stop=True, tile_position=(0, 0))
            b1f = glue.tile([H, 1], f32)
            nc.vector.tensor_add(b1f[:], b1p[:], b1sb[:])

            # broadcast b1' and s to [128,1] via PE selector matmuls
            ps_b = psg_pool.tile([128, 1], f32, tag="psg")
            nc.tensor.matmul(ps_b[:], b64sb[:], b1f[:], start=True,
                             stop=True, tile_position=(0, 0))
            bias128 = wpool.tile([128, 1], f32)
            nc.vector.tensor_copy(bias128[:], ps_b[:])
            ps_s = psg_pool.tile([128, 1], f32, tag="psg")
            nc.tensor.matmul(ps_s[:], b32sb[:], sg[:], start=True,
                             stop=True, tile_position=(0, 0))
            s4 = wpool.tile([128, 1], f32)
            nc.vector.tensor_copy(s4[:], ps_s[:])
            # scale all four W1 copies in place: W1' = diag(s) @ W1
            nc.vector.tensor_scalar(w1sb[:], w1sb[:], s4[:], None,
                                    op0=mybir.AluOpType.mult)

            # ---- pass 2: the MLP (from resident SBUF) ---------------------
            relu = mybir.ActivationFunctionType.Relu

            def _mm1_relu(p):
                xres = xtiles[p]
                hs = []
                for u in range(2):
                    xs = xres[:, FD * u:FD * u + FD]
                    psAB = psab_pool.tile([128, 2 * FD], f32, tag="psAB")
                    # paired mm1: blockdiag(W1',W1') handles two quarters
                    # per matmul; q0q1 h -> cols 0:512, q2q3 -> 512:1024
                    nc.tensor.matmul(psAB[:, 0:FD], w1sb[0:2 * S, :],
                                     xs[0:2 * S, :], start=True, stop=True,
                                     tile_position=(0, 0))
                    nc.tensor.matmul(psAB[:, FD:2 * FD], w1sb[2 * S:128, :],
                                     xs[2 * S:128, :], start=True, stop=True,
                                     tile_position=(64, 0))
                    hU = hpool.tile([128, 2 * FD], f16, tag="hU")
                    # relu(z + b1'): one [128,1024] copy; alternate engines
                    if u == 0:
                        nc.scalar.activation(hU[:], psAB[:], relu,
                                             bias=bias128[:])
                    else:
                        nc.vector.tensor_scalar(hU[:], psAB[:], bias128[:],
                                                0.0, op0=mybir.AluOpType.add,
                                                op1=mybir.AluOpType.max)
                    hs.append(hU)
                return hs

            def _mm2_and_out(p, hs):
                # psC packs both supertiles by PARTITION: u0 -> 0:64,
                # u1 -> 64:128, so the omega copy and DMA run full-width
                psC = psc_pool.tile([128, FD], f32, tag="psC")
                for u in range(2):
                    hU = hs[u]
                    # mm2: hidden-blockdiag W2 -> two quarters' heads per
                    # matmul; supertile u fills psC[64u : 64u+64]
                    nc.tensor.matmul(psC[64 * u:64 * u + 32, :],
                                     w2sb[:, 0:32], hU[:, 0:FD],
                                     start=True, stop=True,
                                     tile_position=(0, 64 * u))
                    nc.tensor.matmul(psC[64 * u + 32:64 * u + 64, :],
                                     w2sb[:, 0:32], hU[:, FD:2 * FD],
                                     start=True, stop=True,
                                     tile_position=(0, 64 * u + 32))
                omega = opool.tile([128, FD], f16, tag="om")
                if p % 2 == 0:
                    nc.vector.tensor_copy(omega[:], psC[:])
                else:
                    nc.scalar.copy(omega[:], psC[:])
                nc.gpsimd.dma_start(oview[p], omega[:])

            # mm2 emitted one pair behind mm1 so the relu latency of pair p
            # hides under pair p+1's mm1 work on the PE
            prev = None
            for p in range(npairs):
                hs = _mm1_relu(p)
                if prev is not None:
                    _mm2_and_out(p - 1, prev)
                prev = hs
            _mm2_and_out(npairs - 1, prev)

    nc.compile()
    return nc


def _get_nc(ncores, npairs):
    key = (ncores, npairs)
    if key not in _NC_CACHE:
        _NC_CACHE[key] = build_nc(ncores, npairs)
    return _NC_CACHE[key]


def make_plan(stacks, mask, gamma, beta, W1, b1, W2, b2, ncores=NCORES):
    """Host-side compaction plan: per-core masked-row indices + capacity."""
    mask = np.asarray(mask)
    idxs = [np.flatnonzero(np.asarray(mask[c]).reshape(-1))
            for c in range(ncores)]
    nmax = max((len(ix) for ix in idxs), default=0)
    npairs = max((nmax + PAIR - 1) // PAIR, 1)
    cnt = max(float(np.asarray(mask, np.float64).sum()), 1.0)
    return {"idxs": idxs, "npairs": npairs, "cnt": cnt}


def make_in_maps(plan, stacks, mask, gamma, beta, W1, b1, W2, b2,
                 ncores=NCORES):
    """Per-core input dicts (host does gather + layout transforms only)."""
    npairs = plan["npairs"]
    rows_c = npairs * PAIR
    inv_cnt = np.float32(1.0 / np.float32(plan["cnt"]))

    svec = np.zeros((S, 4), np.float32)
    svec[:, 0] = np.asarray(gamma, np.float32)
    svec[:, 1] = np.asarray(beta, np.float32)
    svec[:, 2] = inv_cnt

    qm = np.zeros((128, S), np.float32)
    qm[np.arange(128), np.arange(128) % S] = 1.0
    b32 = np.ascontiguousarray(qm.T)              # [32, 128]
    b64 = np.zeros((H, 128), np.float32)
    b64[np.arange(128) % H, np.arange(128)] = 1.0

    w1np = np.asarray(W1, np.float32)
    bd = np.zeros((2 * S, 2 * H), np.float32)     # blockdiag(W1, W1)
    bd[:S, :H] = w1np
    bd[S:, H:] = w1np
    w1f = np.tile(bd, (2, 1)).astype(np.float16)  # [128, 128]
    w2np = np.asarray(W2, np.float32)
    w2f = np.zeros((128, 2 * HEADS), np.float16)  # [[W2,0],[0,W2]] on hidden
    w2f[:H, :HEADS] = w2np.astype(np.float16)
    w2f[H:, HEADS:] = w2np.astype(np.float16)
    b1cc = np.asarray(b1, np.float32).reshape(H, 1)

    in_maps = []
    for c in range(ncores):
        idx = plan["idxs"][c]
        xbuf = np.zeros((rows_c, S), np.float16)
        xbuf[:len(idx)] = np.asarray(stacks[c], np.float32).reshape(-1, S)[idx]
        # row r = ((pair*2 + u)*4 + q)*512 + j ; partition p = q*32 + s
        v = xbuf.reshape(npairs, 2, QS, FD, S)     # [pair, u, q, j, s]
        v = v.transpose(0, 2, 4, 1, 3)             # [pair, q, s, u, j]
        xti = np.ascontiguousarray(v).reshape(npairs, 128, 2 * FD)
        in_maps.append({
            "xt": xti, "w1f": w1f, "w2f": w2f, "w1r": w1np,
            "svec": svec, "b1c": b1cc,
            "qmat": qm, "bm32": b32, "bm64": b64,
        })
    return in_maps


def assemble_output(plan, results, b2, ncores=NCORES):
    npairs = plan["npairs"]
    rows_c = npairs * PAIR
    b2f = np.asarray(b2, np.float32).reshape(1, HEADS)
    outs = []
    for c in range(ncores):
        o = results[c]["out"].astype(np.float32)   # [npairs, 128, 512] fp16
        o = o.reshape(npairs, 2, QS, HEADS, FD)    # [pair, u, q, h, j]
        o = o.transpose(0, 1, 2, 4, 3)             # [pair, u, q, j, h]
        o = np.ascontiguousarray(o).reshape(rows_c, HEADS)
        idx = plan["idxs"][c]
        full = np.zeros((R_FULL, HEADS), np.float32)
        full[idx] = o[:len(idx)] + b2f
        outs.append(full)
    return np.stack(outs)                          # [ncores, R_FULL, 16]


def kernel(stacks, mask, gamma, beta, W1, b1, W2, b2):
    from concourse.bass_utils import run_bass_kernel_spmd

    plan = make_plan(stacks, mask, gamma, beta, W1, b1, W2, b2)
    nc = _get_nc(NCORES, plan["npairs"])
    in_maps = make_in_maps(plan, stacks, mask, gamma, beta, W1, b1, W2, b2)
    res = run_bass_kernel_spmd(nc, in_maps, list(range(NCORES)))
    out = assemble_output(plan, res.results, b2)
    return out.reshape(B, NN, NN, HEADS)


# revision 28
# speedup vs baseline: 2.2947x; 1.1218x over previous
"""AdjStackAttentionWeights kernel for 8 Trainium2 NeuronCores.

Computation: masked BatchNorm (training-mode stats over masked rows of the
whole tensor), normalize, 2-layer MLP (32 -> 64 relu -> 16), mask the output.

Strategy (mask compaction + fully SBUF-resident single pass):
  - Shard batch dim b across the 8 cores (data parallel).
  - ~50% of rows are masked out and produce zero output. The host gathers
    only the masked-in rows per core, pads to a pair-of-supertiles multiple
    (4096 rows), and uploads fp16 in the exact [128, 1024] SBUF tile layout
    (partition p = q*32 + s holds feature s of row-quarter q; free dim is
    two 512-row supertiles). Roughly halves both DMA and PE work.
  - The whole compacted input (~8.5 MiB/core) stays RESIDENT in SBUF: one
    HBM read total.
  - pass 1 (overlapped with the input DMA): per-partition sum via DVE
    tensor_scalar+accum and sum-of-squares via DVE tensor_tensor_reduce
    (fp16 operands -> DVE high-rate modes), folded to [32,2] partials on
    the PE -> AllReduce across the 8 cores.
  - fold: BN scale folded into W1 (W1' = diag(s)@W1); shift becomes a
    per-partition bias b1' = (beta - mean*s)@W1 + b1 applied during relu.
  - pass 2 (from SBUF): per supertile u, mm1 computes all 4 quarters' h into
    one [128, 1024] PSUM tile (two 64-contraction blockdiag(W1',W1')
    matmuls); one relu+bias copy -> fp16 h (ACT for even u, DVE for odd);
    mm2 uses a [128, 32] hidden-blockdiag W2 so ONE matmul yields two
    quarters' 16 heads; both supertiles' heads pack into a full-width
    [128, 512] psC, copied once to fp16 omega and DMA'd out (GpSimd DGE).
    mm2 is emitted one pair behind mm1 so relu latency hides under PE work.
  - b2 and the output mask/scatter are applied on the host (b2 is a
    constant [16] broadcast, same class of host work as the mask multiply).
"""

import numpy as np

B, NN, S, H, HEADS = 8, 512, 32, 64, 16
R_FULL = NN * NN   # 262144 rows per core before compaction
FD = 512           # free-dim elements per supertile quarter
QS = 4             # quarters stacked on the partition axis
ST = QS * FD       # 2048 rows per supertile
PAIR = 2 * ST      # 4096 rows per resident [128, 1024] tile
NCORES = 8
BN_EPS = 1e-5

_NC_CACHE = {}


def build_nc(ncores=NCORES, npairs=33):
    """Build (and bacc-compile) the SPMD bass program for one core."""
    import concourse.bass as bass
    import concourse.tile as tile
    from concourse import bacc, mybir

    f32 = mybir.dt.float32
    f16 = mybir.dt.float16

    nc = bacc.Bacc("TRN2", target_bir_lowering=False, debug=False,
                   num_devices=ncores)

    xt = nc.dram_tensor("xt", [npairs, 128, 2 * FD], f16, kind="ExternalInput")
    # w1f: two stacked copies of blockdiag(W1, W1) [128, 128]
    w1f = nc.dram_tensor("w1f", [128, 2 * H], f16, kind="ExternalInput")
    # w2f: hidden-blockdiag [[W2, 0], [0, W2]] [128, 2*HEADS]
    w2f = nc.dram_tensor("w2f", [128, 2 * HEADS], f16, kind="ExternalInput")
    w1r = nc.dram_tensor("w1r", [S, H], f32, kind="ExternalInput")  # raw W1
    # constant selector matrices for PE-side partition reshuffles
    qmat = nc.dram_tensor("qmat", [128, S], f32, kind="ExternalInput")
    bm32 = nc.dram_tensor("bm32", [S, 128], f32, kind="ExternalInput")
    bm64 = nc.dram_tensor("bm64", [H, 128], f32, kind="ExternalInput")
    svec = nc.dram_tensor("svec", [S, 4], f32, kind="ExternalInput")
    b1c = nc.dram_tensor("b1c", [H, 1], f32, kind="ExternalInput")
    out = nc.dram_tensor("out", [npairs, 128, FD], f16,
                         kind="ExternalOutput")

    xview = xt.ap()
    oview = out.ap()

    with tile.TileContext(nc) as tc:
        with (
            tc.tile_pool(name="wpool", bufs=1) as wpool,
            tc.tile_pool(name="glue", bufs=1) as glue,
            tc.tile_pool(name="bn", bufs=1) as bnpool,
            tc.tile_pool(name="res", bufs=1) as respool,
            tc.tile_pool(name="h", bufs=4) as hpool,
            tc.tile_pool(name="o", bufs=3) as opool,
            tc.tile_pool(name="psAB", bufs=2, space="PSUM") as psab_pool,
            tc.tile_pool(name="psC", bufs=2, space="PSUM") as psc_pool,
            tc.tile_pool(name="psG", bufs=1, space="PSUM") as psg_pool,
            tc.tile_pool(name="dram", bufs=1, space="DRAM") as dpool,
        ):
            # ---- pass 1: DMA-in everything, masked stats ------------------
            # (input DMAs are issued FIRST so the big stream starts at t~0;
            # the small weight/constant DMAs queue behind the first few on
            # SP and still land long before the fold needs them)
            # DVE bn_stats ~1.4us/pair vs ACT accum ~2.6us/pair (two passes
            # plus two 280ns accumulator reads)
            dve_pairs = [p for p in range(npairs) if p % 3 < 2]
            Td = 2 * len(dve_pairs)
            Ta = npairs - len(dve_pairs)
            bnbuf = bnpool.tile([128, 6 * Td], f32)
            # interleaved [sum, sq] per ACT pair -> one strided reduce later
            acc = bnpool.tile([128, max(2 * Ta, 2)], f32)
            sqscr = bnpool.tile([128, 2 * FD], f16)   # discarded squares
            sumscr = bnpool.tile([128, 2 * FD], f16)  # discarded copies
            xtiles = []

            def _weight_dmas():
                w1sb = wpool.tile([128, 2 * H], f16)  # 2x blockdiag(W1, W1)
                nc.sync.dma_start(w1sb[:], w1f[:])
                w2sb = wpool.tile([128, 2 * HEADS], f16)  # hidden-blockdiag
                nc.sync.dma_start(w2sb[:], w2f[:])
                w1rsb = glue.tile([S, H], f32)
                nc.sync.dma_start(w1rsb[:], w1r[:])
                qmsb = glue.tile([128, S], f32)
                nc.sync.dma_start(qmsb[:], qmat[:])
                b32sb = glue.tile([S, 128], f32)
                nc.sync.dma_start(b32sb[:], bm32[:])
                b64sb = glue.tile([H, 128], f32)
                nc.sync.dma_start(b64sb[:], bm64[:])
                svsb = glue.tile([S, 4], f32)
                nc.sync.dma_start(svsb[:], svec[:])
                b1sb = glue.tile([H, 1], f32)
                nc.sync.dma_start(b1sb[:], b1c[:])
                return w1sb, w2sb, w1rsb, qmsb, b32sb, b64sb, svsb, b1sb

            di = ai = 0
            for p in range(npairs):
                xres = respool.tile([128, 2 * FD], f16, tag=f"res{p}")
                xtiles.append(xres)
                nc.sync.dma_start(xres[:], xview[p])
                if p == 2:
                    (w1sb, w2sb, w1rsb, qmsb, b32sb, b64sb, svsb,
                     b1sb) = _weight_dmas()
                if p % 3 < 2:
                    for u in range(2):
                        t = 2 * di + u
                        nc.vector.bn_stats(bnbuf[:, 6 * t:6 * t + 6],
                                           xres[:, FD * u:FD * u + FD])
                    di += 1
                else:
                    nc.scalar.activation(
                        sqscr[:], xres[:],
                        mybir.ActivationFunctionType.Square,
                        accum_out=acc[:, 2 * ai + 1:2 * ai + 2])
                    nc.scalar.activation(
                        sumscr[:], xres[:],
                        mybir.ActivationFunctionType.Identity,
                        accum_out=acc[:, 2 * ai:2 * ai + 1])
                    ai += 1

            # preload the Sqrt activation table now (after the Square /
            # Identity stats ops) so the post-AllReduce fold hits it warm
            sqwarm = glue.tile([1, 1], f32)
            nc.scalar.activation(sqwarm[:], sqscr[0:1, 0:1],
                                 mybir.ActivationFunctionType.Sqrt)

            # convert bn_stats (count, mean, count*var) x {even, odd} and
            # the ACT accumulators into per-partition sum / sumsq
            bnv = bnbuf[:].rearrange("p (t k) -> p t k", k=6)
            means = bnv[:, :, 1:5:3]   # [128, Td, 2] (cols 1 and 4)
            cvars = bnv[:, :, 2:6:3]   # [128, Td, 2] (cols 2 and 5)
            half = float(FD // 2)

            msq = glue.tile([128, 2 * Td], f32)
            nc.vector.tensor_mul(msq[:], means, means)
            sum_means = glue.tile([128, 1], f32)
            nc.vector.tensor_reduce(sum_means[:], means,
                                    axis=mybir.AxisListType.XY,
                                    op=mybir.AluOpType.add)
            sum_msq = glue.tile([128, 1], f32)
            nc.vector.tensor_reduce(sum_msq[:], msq[:],
                                    axis=mybir.AxisListType.X,
                                    op=mybir.AluOpType.add)
            sum_cv = glue.tile([128, 1], f32)
            nc.vector.tensor_reduce(sum_cv[:], cvars,
                                    axis=mybir.AxisListType.XY,
                                    op=mybir.AluOpType.add)
            # ACT lanes: one strided reduce folds [sum, sq] x Ta -> [128, 2]
            pa = glue.tile([128, 2], f32)
            accv = acc[:, 0:2 * Ta].rearrange("p (t k) -> p k t", k=2)
            nc.vector.tensor_reduce(pa[:], accv, axis=mybir.AxisListType.X,
                                    op=mybir.AluOpType.add)
            partials = glue.tile([128, 2], f32)
            # sum(x) = half * sum(means) + act-lane sums
            nc.vector.tensor_scalar(partials[:, 0:1], sum_means[:], half,
                                    pa[:, 0:1], op0=mybir.AluOpType.mult,
                                    op1=mybir.AluOpType.add)
            # sum(x^2) = half * sum(means^2) + sum(count*var) + act-lane sq
            nc.vector.tensor_scalar(partials[:, 1:2], sum_msq[:], half,
                                    sum_cv[:], op0=mybir.AluOpType.mult,
                                    op1=mybir.AluOpType.add)
            nc.vector.tensor_add(partials[:, 1:2], partials[:, 1:2],
                                 pa[:, 1:2])

            # fold the 4 partition quarters on the PE: local = Q.T @ partials
            ps_st = psg_pool.tile([S, 2], f32, tag="psg")
            nc.tensor.matmul(ps_st[:], qmsb[:], partials[:], start=True,
                             stop=True, tile_position=(0, 0))
            local = glue.tile([S, 2], f32)
            nc.vector.tensor_copy(local[:], ps_st[:])

            # ---- AllReduce of [32,2] masked sums across cores -------------
            ar_in = dpool.tile([S, 2], f32)
            ar_out = dpool.tile([S, 2], f32)
            nc.gpsimd.dma_start(ar_in[:], local[:])
            nc.gpsimd.collective_compute(
                "AllReduce",
                mybir.AluOpType.add,
                replica_groups=[list(range(ncores))],
                ins=[ar_in.opt()],
                outs=[ar_out.opt()],
            )
            gst = glue.tile([S, 2], f32)
            nc.gpsimd.dma_start(gst[:], ar_out[:])

            # ---- fold stats into weights ----------------------------------
            # [sum, sumsq] * inv_cnt -> [mean, E[x^2]] in one op
            me = glue.tile([S, 2], f32)
            nc.vector.tensor_scalar(me[:], gst[:], svsb[:, 2:3], None,
                                    op0=mybir.AluOpType.mult)
            var = glue.tile([S, 1], f32)
            nc.vector.tensor_mul(var[:], me[:, 0:1], me[:, 0:1])
            nc.vector.tensor_sub(var[:], me[:, 1:2], var[:])
            nc.vector.tensor_scalar_add(var[:], var[:], BN_EPS)
            recip = glue.tile([S, 1], f32)
            nc.vector.reciprocal(recip[:], var[:])
            rstd = glue.tile([S, 1], f32)
            nc.scalar.activation(rstd[:], recip[:],
                                 mybir.ActivationFunctionType.Sqrt)
            sg = glue.tile([S, 1], f32)
            nc.vector.tensor_mul(sg[:], rstd[:], svsb[:, 0:1])    # s=gamma*rstd
            tv = glue.tile([S, 1], f32)
            nc.vector.tensor_mul(tv[:], me[:, 0:1], sg[:])
            nc.vector.tensor_sub(tv[:], svsb[:, 1:2], tv[:])      # t=beta-mean*s
            # b1' = W1.T @ t + b1
            b1p = psg_pool.tile([H, 1], f32, tag="psg")
            nc.tensor.matmul(b1p[:], w1rsb[:], tv[:], start=True,
                             stop=True, tile_position=(0, 0))
            b1f = glue.tile([H, 1], f32)
            nc.vector.tensor_add(b1f[:], b1p[:], b1sb[:])

            # broadcast b1' and s to [128,1] via PE selector matmuls
            ps_b = psg_pool.tile([128, 1], f32, tag="psg")
            nc.tensor.matmul(ps_b[:], b64sb[:], b1f[:], start=True,
                             stop=True, tile_position=(0, 0))
            bias128 = wpool.tile([128, 1], f32)
            nc.vector.tensor_copy(bias128[:], ps_b[:])
            ps_s = psg_pool.tile([128, 1], f32, tag="psg")
            nc.tensor.matmul(ps_s[:], b32sb[:], sg[:], start=True,
                             stop=True, tile_position=(0, 0))
            s4 = wpool.tile([128, 1], f32)
            nc.vector.tensor_copy(s4[:], ps_s[:])
            # scale all four W1 copies in place: W1' = diag(s) @ W1
            nc.vector.tensor_scalar(w1sb[:], w1sb[:], s4[:], None,
                                    op0=mybir.AluOpType.mult)

            # ---- pass 2: the MLP (from resident SBUF) ---------------------
            relu = mybir.ActivationFunctionType.Relu

            def _mm1_relu(p):
                xres = xtiles[p]
                hs = []
                for u in range(2):
                    xs = xres[:, FD * u:FD * u + FD]
                    psAB = psab_pool.tile([128, 2 * FD], f32, tag="psAB")
                    # paired mm1: blockdiag(W1',W1') handles two quarters
                    # per matmul; q0q1 h -> cols 0:512, q2q3 -> 512:1024
                    nc.tensor.matmul(psAB[:, 0:FD], w1sb[0:2 * S, :],
                                     xs[0:2 * S, :], start=True, stop=True,
                                     tile_position=(0, 0))
                    nc.tensor.matmul(psAB[:, FD:2 * FD], w1sb[2 * S:128, :],
                                     xs[2 * S:128, :], start=True, stop=True,
                                     tile_position=(64, 0))
                    hU = hpool.tile([128, 2 * FD], f16, tag="hU")
                    # relu(z + b1'): one [128,1024] copy; alternate engines
                    if u == 0:
                        nc.scalar.activation(hU[:], psAB[:], relu,
                                             bias=bias128[:])
                    else:
                        nc.vector.tensor_scalar(hU[:], psAB[:], bias128[:],
                                                0.0, op0=mybir.AluOpType.add,
                                                op1=mybir.AluOpType.max)
                    hs.append(hU)
                return hs

            def _mm2_and_out(p, hs):
                # psC packs both supertiles by PARTITION: u0 -> 0:64,
                # u1 -> 64:128, so the omega copy and DMA run full-width
                psC = psc_pool.tile([128, FD], f32, tag="psC")
                for u in range(2):
                    hU = hs[u]
                    # mm2: hidden-blockdiag W2 -> two quarters' heads per
                    # matmul; supertile u fills psC[64u : 64u+64]
                    nc.tensor.matmul(psC[64 * u:64 * u + 32, :],
                                     w2sb[:, 0:32], hU[:, 0:FD],
                                     start=True, stop=True,
                                     tile_position=(0, 64 * u))
                    nc.tensor.matmul(psC[64 * u + 32:64 * u + 64, :],
                                     w2sb[:, 0:32], hU[:, FD:2 * FD],
                                     start=True, stop=True,
                                     tile_position=(0, 64 * u + 32))
                omega = opool.tile([128, FD], f16, tag="om")
                if p % 2 == 0:
                    nc.vector.tensor_copy(omega[:], psC[:])
                else:
                    nc.scalar.copy(omega[:], psC[:])
                nc.gpsimd.dma_start(oview[p], omega[:])

            # mm2 emitted one pair behind mm1 so the relu latency of pair p
            # hides under pair p+1's mm1 work on the PE
            prev = None
            for p in range(npairs):
                hs = _mm1_relu(p)
                if prev is not None:
                    _mm2_and_out(p - 1, prev)
                prev = hs
            _mm2_and_out(npairs - 1, prev)

    nc.compile()
    return nc


def _get_nc(ncores, npairs):
    key = (ncores, npairs)
    if key not in _NC_CACHE:
        _NC_CACHE[key] = build_nc(ncores, npairs)
    return _NC_CACHE[key]


def make_plan(stacks, mask, gamma, beta, W1, b1, W2, b2, ncores=NCORES):
    """Host-side compaction plan: per-core masked-row indices + capacity."""
    mask = np.asarray(mask)
    idxs = [np.flatnonzero(np.asarray(mask[c]).reshape(-1))
            for c in range(ncores)]
    nmax = max((len(ix) for ix in idxs), default=0)
    npairs = max((nmax + PAIR - 1) // PAIR, 1)
    cnt = max(float(np.asarray(mask, np.float64).sum()), 1.0)
    return {"idxs": idxs, "npairs": npairs, "cnt": cnt}


def make_in_maps(plan, stacks, mask, gamma, beta, W1, b1, W2, b2,
                 ncores=NCORES):
    """Per-core input dicts (host does gather + layout transforms only)."""
    npairs = plan["npairs"]
    rows_c = npairs * PAIR
    inv_cnt = np.float32(1.0 / np.float32(plan["cnt"]))

    svec = np.zeros((S, 4), np.float32)
    svec[:, 0] = np.asarray(gamma, np.float32)
    svec[:, 1] = np.asarray(beta, np.float32)
    svec[:, 2] = inv_cnt

    qm = np.zeros((128, S), np.float32)
    qm[np.arange(128), np.arange(128) % S] = 1.0
    b32 = np.ascontiguousarray(qm.T)              # [32, 128]
    b64 = np.zeros((H, 128), np.float32)
    b64[np.arange(128) % H, np.arange(128)] = 1.0

    w1np = np.asarray(W1, np.float32)
    bd = np.zeros((2 * S, 2 * H), np.float32)     # blockdiag(W1, W1)
    bd[:S, :H] = w1np
    bd[S:, H:] = w1np
    w1f = np.tile(bd, (2, 1)).astype(np.float16)  # [128, 128]
    w2np = np.asarray(W2, np.float32)
    w2f = np.zeros((128, 2 * HEADS), np.float16)  # [[W2,0],[0,W2]] on hidden
    w2f[:H, :HEADS] = w2np.astype(np.float16)
    w2f[H:, HEADS:] = w2np.astype(np.float16)
    b1cc = np.asarray(b1, np.float32).reshape(H, 1)

    in_maps = []
    for c in range(ncores):
        idx = plan["idxs"][c]
        xbuf = np.zeros((rows_c, S), np.float16)
        xbuf[:len(idx)] = np.asarray(stacks[c], np.float32).reshape(-1, S)[idx]
        # row r = ((pair*2 + u)*4 + q)*512 + j ; partition p = q*32 + s
        v = xbuf.reshape(npairs, 2, QS, FD, S)     # [pair, u, q, j, s]
        v = v.transpose(0, 2, 4, 1, 3)             # [pair, q, s, u, j]
        xti = np.ascontiguousarray(v).reshape(npairs, 128, 2 * FD)
        in_maps.append({
            "xt": xti, "w1f": w1f, "w2f": w2f, "w1r": w1np,
            "svec": svec, "b1c": b1cc,
            "qmat": qm, "bm32": b32, "bm64": b64,
        })
    return in_maps


def assemble_output(plan, results, b2, ncores=NCORES):
    npairs = plan["npairs"]
    rows_c = npairs * PAIR
    b2f = np.asarray(b2, np.float32).reshape(1, HEADS)
    outs = []
    for c in range(ncores):
        o = results[c]["out"].astype(np.float32)   # [npairs, 128, 512] fp16
        o = o.reshape(npairs, 2, QS, HEADS, FD)    # [pair, u, q, h, j]
        o = o.transpose(0, 1, 2, 4, 3)             # [pair, u, q, j, h]
        o = np.ascontiguousarray(o).reshape(rows_c, HEADS)
        idx = plan["idxs"][c]
        full = np.zeros((R_FULL, HEADS), np.float32)
        full[idx] = o[:len(idx)] + b2f
        outs.append(full)
    return np.stack(outs)                          # [ncores, R_FULL, 16]


def kernel(stacks, mask, gamma, beta, W1, b1, W2, b2):
    from concourse.bass_utils import run_bass_kernel_spmd

    plan = make_plan(stacks, mask, gamma, beta, W1, b1, W2, b2)
    nc = _get_nc(NCORES, plan["npairs"])
    in_maps = make_in_maps(plan, stacks, mask, gamma, beta, W1, b1, W2, b2)
    res = run_bass_kernel_spmd(nc, in_maps, list(range(NCORES)))
    out = assemble_output(plan, res.results, b2)
    return out.reshape(B, NN, NN, HEADS)
